# revision 70
# baseline (speedup 1.0000x reference)
"""GAT (gnn_message_passing) Trainium2 Bass kernel — 8-core SPMD.

Contract: kernel(**inputs) -> np.ndarray with FULL inputs / FULL output.
Self-contained: hardcodes shapes; only imports the container's concourse stack.

Design:
- Stage A: per-core h = (x @ fb) @ Wg (x/fb in fp8) plus attention dots ->
  local 256B node rows [h fp8(128) | a_s bf16(4) | a_d bf16(4) | pad].
- AllGather (unmerged row-structured APs) shares the node table.
- Edge phase, grouped by GRP destination slots per gather batch, with the
  next group's gathers issued one group ahead (software pipelining):
  * 136B/row gathers of source rows (h + a_s), in two int16 index halves,
  * an 8B/row gather of per-edge destination attention (a_d),
  * batched indicator builds and ex = exp(prelu(a_s+a_d)) with duplicated
    bf16 pairs so DVE ops hit the 2x perf mode,
  * msg = h * ex (fp8 x bf16 -> bf16; one head per region on gpsimd),
  * per-chunk indicator matmuls accumulate numerators+denominators in PSUM,
  * normalize (Act scale-copies), ELU via transpose, fused MLP + softmax.
- Destination tiles are assigned to (core, slot) by balanced ranking so the
  SPMD max-padding across cores stays small.
"""
import sys

for _p in ("/opt/trn_rl_repo", "/root/.axon_site/_ro/trn_rl_repo"):
    if _p not in sys.path:
        sys.path.append(_p)

import numpy as np

# ---------------- problem constants (hardcoded per contract) ----------------
N = 50000
NF = 513
NFP = 640            # padded feature dim (5 * 128)
NMEL = 128
H, C = 4, 32
HC = H * C           # 128
E = 800000
NEG_ATT = 0.2
NEG_MLP = 0.01

NCORES = 8
TPC = 49             # tiles (slots) per core
NT = 128             # nodes per tile
NPC = TPC * NT       # 6272 nodes per core
NPAD = NCORES * NPC  # 50176
NTILES_G = NPAD // NT  # 392
RW = 256             # Hfull row elems (bf16 -> 512 B)
ADR = 128            # adrep row elems (bf16 -> 256 B row stride)
SPLIT = 32768        # max int16 gather index + 1
BASE_B = NPAD - SPLIT  # 17408; group-B gathers read Hfull[BASE_B:]
GRP = 6              # slots per gather group

_CACHE = {}


def _to_fp8(a):
    import ml_dtypes
    return np.asarray(a, dtype=np.float32).astype(ml_dtypes.float8_e4m3fn)


def _to_bf16(a):
    """f32 -> bf16 (round-to-nearest-even)."""
    try:
        import ml_dtypes
        return np.asarray(a, dtype=np.float32).astype(ml_dtypes.bfloat16)
    except ImportError:
        x = np.ascontiguousarray(a, dtype=np.float32).view(np.uint32)
        rounded = (((x >> 16) + ((x >> 15) & 1)) & 0xFFFF).astype(np.uint16)
        return rounded


def _prep(edge_index):
    """Host-side edge preprocessing. Returns per-core index/metadata arrays."""
    src = np.asarray(edge_index[0], dtype=np.int64)
    dst = np.asarray(edge_index[1], dtype=np.int64)
    loop = np.arange(N, dtype=np.int64)
    src = np.concatenate([src, loop])
    dst = np.concatenate([dst, loop])

    # ---- balanced tile -> (core, slot) assignment ----
    tile_of = dst // NT
    tcnt = np.bincount(tile_of, minlength=NTILES_G)
    order = np.argsort(-tcnt, kind="stable")      # rank r -> tile
    # slot r//8 gets ranks r with r%8 as core
    tile_core = np.empty(NTILES_G, dtype=np.int64)
    tile_slot = np.empty(NTILES_G, dtype=np.int64)
    for r, t in enumerate(order):
        tile_slot[t] = r // NCORES
        tile_core[t] = r % NCORES

    # node -> permuted global row
    nodes = np.arange(NPAD, dtype=np.int64)
    n_tile = nodes // NT
    perm_row = tile_core[n_tile] * NPC + tile_slot[n_tile] * NT + nodes % NT
    # node_order[k, local] = node id
    node_order = np.empty((NCORES, NPC), dtype=np.int64)
    node_order[perm_row // NPC, perm_row % NPC] = nodes

    # ---- per-edge core/slot/half ----
    ecore = tile_core[tile_of]
    eslot = tile_slot[tile_of]
    prow = perm_row[src]
    half = (prow >= SPLIT).astype(np.int64)
    srow = np.where(half == 1, prow - BASE_B, prow)
    dloc = dst % NT

    # sort per (core, slot, half, dst)
    so = np.lexsort((src, dloc, half, eslot, ecore))
    ecore, eslot, half, srow, dloc = (a[so] for a in (ecore, eslot, half, srow, dloc))

    # counts per (core, slot, half)
    cnt = np.zeros((NCORES, TPC, 2), dtype=np.int64)
    np.add.at(cnt, (ecore, eslot, half), 1)
    starts = np.zeros((NCORES, TPC, 2), dtype=np.int64)
    starts.reshape(-1)[1:] = np.cumsum(cnt.reshape(-1))[:-1]

    # chunks per (slot, half): max over cores
    cpt = np.maximum(1, -(-cnt.max(axis=0) // NT))   # [TPC, 2]

    ngrp = -(-TPC // GRP)
    # chunk order: per group g: A-chunks of slots g*4.., then B-chunks
    slot_chunk_off = np.zeros((TPC, 2), dtype=np.int64)   # abs chunk idx of each (s,half) run
    grp_a0 = np.zeros(ngrp, dtype=np.int64)
    grp_b0 = np.zeros(ngrp, dtype=np.int64)
    grp_end = np.zeros(ngrp, dtype=np.int64)
    co = 0
    for g in range(ngrp):
        slots = range(g * GRP, min((g + 1) * GRP, TPC))
        grp_a0[g] = co
        for s in slots:
            slot_chunk_off[s, 0] = co
            co += cpt[s, 0]
        grp_b0[g] = co
        for s in slots:
            slot_chunk_off[s, 1] = co
            co += cpt[s, 1]
        grp_end[g] = co
    TOTC = co
    TOTIDX = TOTC * NT

    # fill per-chunk per-core tables (chunk-major layout)
    src_rel = np.zeros((NCORES, TOTC, NT), dtype=np.int64)
    ad_idx = np.zeros((NCORES, TOTC, NT), dtype=np.int64)
    dloc_all = np.zeros((NCORES, TOTC, NT), dtype=np.int64)
    valid = np.zeros((NCORES, TOTC, NT), dtype=bool)
    for k in range(NCORES):
        for s in range(TPC):
            for hf in range(2):
                c0 = slot_chunk_off[s, hf]
                nch = int(cpt[s, hf])
                st, cn = starts[k, s, hf], int(cnt[k, s, hf])
                src_rel[k, c0:c0 + nch].reshape(-1)[:cn] = srow[st:st + cn]
                ad_idx[k, c0:c0 + nch].reshape(-1)[:cn] = (
                    eslot[st:st + cn] * NT + dloc[st:st + cn])
                dloc_all[k, c0:c0 + nch].reshape(-1)[:cn] = dloc[st:st + cn]
                valid[k, c0:c0 + nch].reshape(-1)[:cn] = True

    assert src_rel.min() >= 0 and src_rel.max() < SPLIT

    # windows per chunk (union over cores): narrowest of 32 at 32-aligned
    # offset <= 64, else 64 at {0, 64}, else full 128. PSUM matmul outputs
    # may start at partitions {0, 32, 64}.
    woff = np.zeros(TOTC, dtype=np.int64)
    wlen = np.full(TOTC, 128, dtype=np.int64)
    for c in range(TOTC):
        v = valid[:, c, :]
        if v.any():
            dl = dloc_all[:, c, :][v]
            lo, hi = int(dl.min()), int(dl.max())
            wo32 = min((lo // 32) * 32, 64)
            if hi < wo32 + 32:
                woff[c] = wo32
                wlen[c] = 32
            else:
                wo = 0 if lo < 64 else 64
                if hi < wo + 64:
                    woff[c] = wo
                    wlen[c] = 64

    # chunk -> owning slot (before region reorder)
    slot_of_chunk = np.zeros(TOTC, dtype=np.int64)
    for s in range(TPC):
        for hf in range(2):
            c0, nch = int(slot_chunk_off[s, hf]), int(cpt[s, hf])
            slot_of_chunk[c0:c0 + nch] = s

    # reorder chunks within each group REGION by wlen desc (stable) so one
    # batched indicator build covers each wl class; track per-slot lists.
    runs_grp = {}
    for g in range(ngrp):
        for r, (r0, r1) in enumerate(((int(grp_a0[g]), int(grp_b0[g])),
                                      (int(grp_b0[g]), int(grp_end[g])))):
            sl = slice(r0, r1)
            ro = np.argsort(-wlen[sl], kind="stable")
            for arr in (woff, wlen, slot_of_chunk):
                arr[sl] = arr[sl][ro]
            for arr in (src_rel, ad_idx, dloc_all, valid):
                arr[:, sl] = arr[:, sl][:, ro]
            rl = []
            i = r0
            while i < r1:
                w = int(wlen[i])
                j = i
                while j < r1 and int(wlen[j]) == w:
                    j += 1
                rl.append((w, i - r0, j - i))
                i = j
            runs_grp[(g, r)] = rl

    # per-slot chunk lists: (region, rel_idx, ind_col, woff, wlen)
    slot_chunks = {s: [] for s in range(TPC)}
    wcol = np.zeros(TOTC, dtype=np.int64)   # ind-arena col offset per chunk
    WSUM_A = WSUM_B = 0
    for g in range(ngrp):
        for r, (r0, r1) in enumerate(((int(grp_a0[g]), int(grp_b0[g])),
                                      (int(grp_b0[g]), int(grp_end[g])))):
            off = 0
            for c in range(r0, r1):
                wcol[c] = off
                off += int(wlen[c])
            if r == 0:
                WSUM_A = max(WSUM_A, off)
            else:
                WSUM_B = max(WSUM_B, off)
            for c in range(r0, r1):
                s = int(slot_of_chunk[c])
                slot_chunks[s].append((r, c - r0, int(wcol[c]), int(woff[c]),
                                       int(wlen[c])))
    for s in range(TPC):
        slot_chunks[s].sort(key=lambda t: -t[4])

    dst_rel = np.full((NCORES, TOTC, NT), 999.0, dtype=np.float32)
    for k in range(NCORES):
        dr = dloc_all[k] - woff[None, :].T
        dst_rel[k][valid[k]] = dr[valid[k]].astype(np.float32)

    # wrapped int16 index layout: [128, TOTIDX//16]
    def wrap(a):
        fl = a.reshape(NCORES, TOTIDX)
        w = fl.reshape(NCORES, TOTIDX // 16, 16).transpose(0, 2, 1)
        return np.tile(w, (1, 8, 1)).astype(np.int16)

    src_w = wrap(src_rel)
    ad_w = wrap(ad_idx)
    # dcol duplicated pairs, bf16: [NCORES, 128, TOTC, 2]
    dcol2 = np.repeat(dst_rel.transpose(0, 2, 1)[:, :, :, None], 2, axis=3)

    meta = {"cpt": cpt, "woff": woff, "wlen": wlen, "TOTC": TOTC,
            "TOTIDX": TOTIDX, "runs_grp": runs_grp, "wcol": wcol,
            "slot_chunks": slot_chunks, "slot_off": slot_chunk_off,
            "grp_a0": grp_a0, "grp_b0": grp_b0, "grp_end": grp_end,
            "ngrp": ngrp, "WSUM_A": WSUM_A, "WSUM_B": WSUM_B}
    return src_w, ad_w, dcol2, node_order, meta


def _build(meta):
    import concourse.bass as bass
    import concourse.bacc as bacc
    import concourse.mybir as mybir
    import concourse.tile as tile

    f32 = mybir.dt.float32
    bf16 = mybir.dt.bfloat16
    fp8 = mybir.dt.float8e4
    i16 = mybir.dt.int16
    AF = mybir.ActivationFunctionType
    OP = mybir.AluOpType

    cpt, woff, wlen = meta["cpt"], meta["woff"], meta["wlen"]
    TOTC, TOTIDX = meta["TOTC"], meta["TOTIDX"]
    runs_grp, slot_off = meta["runs_grp"], meta["slot_off"]
    slot_chunks, wcol = meta["slot_chunks"], meta["wcol"]
    grp_a0, grp_b0, grp_end = meta["grp_a0"], meta["grp_b0"], meta["grp_end"]
    ngrp = meta["ngrp"]

    GA_MAX = int(max(int(grp_b0[g] - grp_a0[g]) for g in range(ngrp)))
    WSUM_A, WSUM_B = meta["WSUM_A"], meta["WSUM_B"]
    GB_MAX = int(max(int(grp_end[g] - grp_b0[g]) for g in range(ngrp)))
    GT_MAX = int(max(int(grp_end[g] - grp_a0[g]) for g in range(ngrp)))
    TOT_MAX = int((cpt[:, 0] + cpt[:, 1]).max())

    nc = bacc.Bacc("TRN2", target_bir_lowering=False, debug=False)

    # ---- I/O ----
    xT_in = nc.dram_tensor("xT_in", [NFP, NPC], fp8, kind="ExternalInput")
    idx_src = nc.dram_tensor("idx_src", [128, TOTIDX // 16], i16, kind="ExternalInput")
    idx_ad = nc.dram_tensor("idx_ad", [128, TOTIDX // 16], i16, kind="ExternalInput")
    dcol_d = nc.dram_tensor("dcol2", [128, TOTC * 2], bf16, kind="ExternalInput")
    fb_p = nc.dram_tensor("fb_p", [NFP, NMEL], fp8, kind="ExternalInput")
    Wg_d = nc.dram_tensor("Wg", [NMEL, HC], f32, kind="ExternalInput")
    Wg_bf_d = nc.dram_tensor("Wg_bf", [NMEL, HC], bf16, kind="ExternalInput")
    attb_s = nc.dram_tensor("attb_s", [HC, 4], f32, kind="ExternalInput")
    attb_d = nc.dram_tensor("attb_d", [HC, 4], f32, kind="ExternalInput")
    bias_col_d = nc.dram_tensor("bias_col", [128, 1], f32, kind="ExternalInput")
    W1_d = nc.dram_tensor("W1", [HC, 256], bf16, kind="ExternalInput")
    b1_d = nc.dram_tensor("b1", [128, 2], f32, kind="ExternalInput")
    W2_d = nc.dram_tensor("W2", [256, HC], bf16, kind="ExternalInput")
    b2_d = nc.dram_tensor("b2", [128, 1], f32, kind="ExternalInput")
    W3_d = nc.dram_tensor("W3", [HC, 10], bf16, kind="ExternalInput")
    b3_d = nc.dram_tensor("b3", [128, 1], f32, kind="ExternalInput")
    eye_d = nc.dram_tensor("eye", [128, 128], f32, kind="ExternalInput")
    iota_d = nc.dram_tensor("iota", [128, 128], bf16, kind="ExternalInput")
    ones_d = nc.dram_tensor("ones", [128, 16], bf16, kind="ExternalInput")
    outT = nc.dram_tensor("outT", [10, NPC], f32, kind="ExternalOutput")

    core_ids = list(range(NCORES))

    def small_gather(g, out_ap, in_ap, idxs_ap, num_idxs, elem_size,
                     stride_b256):
        """Hand-rolled InstDMAGatherAnt allowing elem < 256B (stride stays a
        256B multiple, which is the actual ISA constraint)."""
        _in_ap = g.lower_ap_dma(in_ap, for_custom_bir_dma=True)
        return g.add_instruction(
            mybir.InstDMAGatherAnt(
                name=g.bass.get_next_instruction_name(),
                ins=[*_in_ap, g.lower_ap(idxs_ap),
                     g.lower_val_access(g.to_reg(num_idxs))],
                outs=[g.lower_ap(out_ap)],
                transpose=False,
                num_idxs=num_idxs,
                elem_size=elem_size,
                stride_bytes_256=stride_b256,
                gen_mode=0,
                single_packet=False,
                queue_num=0,
                sbuf_tokens_per_rank=0,
                sbuf_free_dim_per_rank=0,
                sbuf_free_dim_pad_per_rank=0,
                sbuf_byte_offset=0,
            ))

    with tile.TileContext(nc) as tc:
        with (
            tc.tile_pool(name="dram", bufs=1, space="DRAM") as dpool,
            tc.tile_pool(name="const", bufs=1) as cpool,
        ):
            Hext_loc = dpool.tile([NPC, RW], fp8)
            Hfull = dpool.tile([NPAD, RW], fp8, addr_space="Shared")

            # ---- constants to SBUF ----
            fb_t = cpool.tile([128, 5, NMEL], fp8)
            nc.sync.dma_start(fb_t[:], fb_p.rearrange("(b p) m -> p b m", p=128))
            Wg_t = cpool.tile([128, HC], f32)
            nc.sync.dma_start(Wg_t[:], Wg_d[:])
            Wg_bf = cpool.tile([128, HC], bf16)
            nc.sync.dma_start(Wg_bf[:], Wg_bf_d[:])
            atts_t = cpool.tile([128, 4], f32)
            nc.sync.dma_start(atts_t[:], attb_s[:])
            attd_t = cpool.tile([128, 4], f32)
            nc.sync.dma_start(attd_t[:], attb_d[:])
            bias_col = cpool.tile([128, 1], f32)
            nc.sync.dma_start(bias_col[:], bias_col_d[:])
            W1_t = cpool.tile([128, 256], bf16)
            nc.sync.dma_start(W1_t[:], W1_d[:])
            b1_t = cpool.tile([128, 2], f32)
            nc.sync.dma_start(b1_t[:], b1_d[:])
            W2_t = cpool.tile([128, 2, HC], bf16)
            nc.sync.dma_start(W2_t[:], W2_d.rearrange("(b p) m -> p b m", p=128))
            b2_t = cpool.tile([128, 1], f32)
            nc.sync.dma_start(b2_t[:], b2_d[:])
            W3_t = cpool.tile([128, 10], bf16)
            nc.sync.dma_start(W3_t[:], W3_d[:])
            b3_t = cpool.tile([128, 1], f32)
            nc.sync.dma_start(b3_t[:], b3_d[:])
            eye_t = cpool.tile([128, 128], f32)
            nc.sync.dma_start(eye_t[:], eye_d[:])
            zrow_t = cpool.tile([1, 136], bf16)
            nc.vector.memset(zrow_t[:], 0.0)
            orow_t = cpool.tile([1, 128], bf16)
            nc.vector.memset(orow_t[:], 1.0)
            iota_t = cpool.tile([128, 128], bf16)
            nc.sync.dma_start(iota_t[:], iota_d[:])
            ones_t = cpool.tile([128, 16], bf16)
            nc.sync.dma_start(ones_t[:], ones_d[:])
            dcol_t = cpool.tile([128, TOTC * 2], bf16)
            nc.scalar.dma_start(dcol_t[:], dcol_d[:])

            # WgT, Wgatt (h -> [h | a_s | a_d] projection columns)
            WgT_t = cpool.tile([128, 128], f32)
            WgA_t = cpool.tile([128, HC + 8], bf16)
            with tc.tile_pool(name="cpsum", bufs=1, space="PSUM") as cpsum:
                WgT_ps = cpsum.tile([128, 128], f32)
                nc.tensor.transpose(WgT_ps[:], Wg_t[:], eye_t[:])
                nc.vector.tensor_copy(WgT_t[:], WgT_ps[:])
                Wgatt_ps = cpsum.tile([128, 8], f32)
                nc.tensor.matmul(Wgatt_ps[:, 0:4], WgT_t[:], atts_t[:])
                nc.tensor.matmul(Wgatt_ps[:, 4:8], WgT_t[:], attd_t[:])
                nc.vector.tensor_copy(WgA_t[:, 0:HC], Wg_bf[:])
                nc.vector.tensor_copy(WgA_t[:, HC:HC + 8], Wgatt_ps[:])

            # ================= stage A =================
            with (
                tc.tile_pool(name="sa_sb", bufs=4) as sa,
                tc.tile_pool(name="sa_ps", bufs=3, space="PSUM") as saps,
                tc.tile_pool(name="sa_ps1", bufs=4, space="PSUM") as saps1,
            ):
                xT8 = None
                for g0 in range(0, TPC, 4):
                    gsz = min(4, TPC - g0)
                    gn = gsz * NT
                    if g0 % 8 == 0:
                        x8n = min(8, TPC - g0) * NT
                        xT8 = sa.tile([128, 5, 8 * NT], fp8, tag="xT", bufs=2)
                        nc.sync.dma_start(
                            xT8[:, :, 0:x8n],
                            xT_in.rearrange("(b p) n -> p b n", p=128)[
                                :, :, g0 * NT:g0 * NT + x8n])
                        xoff = 0
                    else:
                        xoff = 4 * NT
                    h1T_ps = saps.tile([128, 4 * NT], f32, tag="h1T")
                    for b in range(5):
                        nc.tensor.matmul(
                            h1T_ps[:, 0:gn],
                            fb_t[:, b, :],
                            xT8[:, b, xoff:xoff + gn],
                            start=(b == 0), stop=(b == 4))
                    h1T = sa.tile([128, 4 * NT], bf16, tag="h1Ts")
                    nc.vector.tensor_copy(h1T[:, 0:gn], h1T_ps[:, 0:gn])
                    hrow4 = sa.tile([128, 4, RW], fp8, tag="hrow4")
                    for u in range(gsz):
                        h_ps = saps1.tile([128, HC + 8], f32, tag="hps")
                        lhs = h1T[:, u * NT:(u + 1) * NT]
                        nc.tensor.matmul(h_ps[:, 0:HC + 8], lhs, WgA_t[:])
                        nc.scalar.activation(hrow4[:, u, 0:HC],
                                             h_ps[:, 0:HC], AF.Copy)
                        nc.scalar.activation(
                            hrow4[:, u, HC:HC + 16].bitcast(bf16),
                            h_ps[:, HC:HC + 8], AF.Copy)
                    nc.sync.dma_start(
                        Hext_loc[g0 * NT:g0 * NT + gn, 0:HC + 16].rearrange(
                            "(u p) c -> p u c", p=128),
                        hrow4[:, 0:gsz, 0:HC + 16])

                # AllGather with unmerged (opt=False) row-major APs.
                g = nc.gpsimd
                g.add_instruction(
                    mybir.InstCollectiveCompute(
                        name=f"I-{g.bass.next_id()}",
                        kind="AllGather",
                        op=mybir.AluOpType.bypass,
                        replica_groups=[core_ids],
                        ins=[g.lower_ap(Hext_loc[:], opt=False)],
                        outs=[g.lower_ap(Hfull[:], opt=False)],
                        unique_tensors="No",
                        cc_dim="Partition",
                    ))

            # ================= edge phase + MLP =================
            with (
                tc.tile_pool(name="eg_g", bufs=2) as egg,
                tc.tile_pool(name="eg_sb", bufs=3) as egs,
                tc.tile_pool(name="eg_ind", bufs=2) as egi,
                tc.tile_pool(name="eg_acc", bufs=3, space="PSUM") as egacc,
                tc.tile_pool(name="eg_tp", bufs=2, space="PSUM") as egtp,
                tc.tile_pool(name="mlp_sb", bufs=2) as msb,
                tc.tile_pool(name="mlp_ps", bufs=1, space="PSUM") as mps,
            ):
                actT4 = None
                mgsz = 4

                def issue_gathers(gidx):
                    a0, b0, e0 = (int(grp_a0[gidx]), int(grp_b0[gidx]),
                                  int(grp_end[gidx]))
                    nA, nB, nT_ = b0 - a0, e0 - b0, e0 - a0
                    isg = egg.tile([128, GT_MAX * 8], i16, tag="isg", bufs=3,
                                   name=f"isg_{gidx}")
                    nc.scalar.dma_start(isg[:, 0:nT_ * 8],
                                        idx_src[:, a0 * 8:e0 * 8])
                    adg = egg.tile([128, GT_MAX * 8], i16, tag="adg", bufs=3,
                                   name=f"adg_{gidx}")
                    nc.scalar.dma_start(adg[:, 0:nT_ * 8],
                                        idx_ad[:, a0 * 8:e0 * 8])
                    gAt = egg.tile([128, GA_MAX, HC + 8], fp8, tag="gA",
                                   bufs=3, name=f"gA_{gidx}")
                    small_gather(nc.gpsimd, gAt[:, 0:nA, :],
                                 Hfull[:, 0:HC + 8],
                                 isg[:, 0:nA * 8],
                                 num_idxs=nA * NT, elem_size=HC + 8,
                                 stride_b256=1)
                    adt = egg.tile([128, GT_MAX, 8], fp8, tag="adt", bufs=3,
                                   name=f"adt_{gidx}")
                    small_gather(nc.gpsimd, adt[:, 0:nT_, :],
                                 Hext_loc[:, HC + 8:HC + 16],
                                 adg[:, 0:nT_ * 8],
                                 num_idxs=nT_ * NT, elem_size=8, stride_b256=1)
                    gBt = egg.tile([128, GB_MAX, HC + 8], fp8, tag="gB",
                                   bufs=3, name=f"gB_{gidx}")
                    small_gather(nc.gpsimd, gBt[:, 0:nB, :],
                                 Hfull[BASE_B:NPAD, 0:HC + 8],
                                 isg[:, nA * 8:nT_ * 8],
                                 num_idxs=nB * NT, elem_size=HC + 8,
                                 stride_b256=1)
                    return gAt, gBt, adt

                from collections import deque
                pq = deque([issue_gathers(0)])
                if ngrp > 1:
                    pq.append(issue_gathers(1))
                for gidx in range(ngrp):
                    s0 = gidx * GRP
                    slots = list(range(s0, min(s0 + GRP, TPC)))
                    a0, b0, e0 = (int(grp_a0[gidx]), int(grp_b0[gidx]),
                                  int(grp_end[gidx]))
                    nA, nB, nT_ = b0 - a0, e0 - b0, e0 - a0
                    gAt, gBt, adt = pq.popleft()
                    if gidx + 2 < ngrp:
                        pq.append(issue_gathers(gidx + 2))

                    # ---- per-region batched attention prep ----
                    gsA = egg.tile([128, GA_MAX, 136], bf16, tag="gsA",
                                   bufs=3, name=f"gsA_{gidx}")
                    gsB = egg.tile([128, GB_MAX, 136], bf16, tag="gsB",
                                   bufs=2, name=f"gsB_{gidx}")
                    regions = [(gAt, nA, 0, 0, gsA), (gBt, nB, b0 - a0, 1, gsB)]
                    inds = []
                    # phase 1: indicator builds (constants only) + a_s+a_d,
                    # prelu, exp for both regions
                    for (gt, nreg, adoff, r, gts) in regions:
                        ind = egi.tile(
                            [128, WSUM_A if r == 0 else WSUM_B], bf16,
                            tag="indA" if r == 0 else "indB",
                            name=f"ind_{gidx}_{r}")
                        inds.append(ind)
                        if nreg == 0:
                            continue
                        roff = a0 if r == 0 else b0
                        for (wl, r0, rl) in runs_grp[(gidx, r)]:
                            i0 = bass.AP(
                                iota_t.tensor, iota_t.offset,
                                [iota_t.ap[0], [0, rl], [2, wl // 2], [1, 2]])
                            i1 = bass.AP(
                                dcol_t.tensor,
                                dcol_t.offset + (roff + r0) * 2,
                                [dcol_t.ap[0], [2, rl], [0, wl // 2], [1, 2]])
                            o0 = bass.AP(
                                ind.tensor,
                                ind.offset + int(wcol[roff + r0]),
                                [ind.ap[0], [wl, rl], [2, wl // 2], [1, 2]])
                            nc.vector.tensor_tensor(o0, i0, i1, OP.is_equal)
                        # t = a_s + a_d (bf16 views of fp8 tiles)
                        ts = egs.tile([128, GA_MAX, 4], bf16, tag="ts",
                                      name=f"ts_{gidx}_{r}")
                        nc.vector.tensor_tensor(
                            ts[:, 0:nreg, :],
                            gt[:, 0:nreg, 128:136].bitcast(bf16),
                            adt[:, adoff:adoff + nreg, :].bitcast(bf16),
                            OP.add)
                        nc.scalar.activation(ts[:, 0:nreg, :], ts[:, 0:nreg, :],
                                             AF.Prelu, alpha=NEG_ATT)
                        # exp, duplicated pairs -> gts cols 128:136 (bf16)
                        ts_in = bass.AP(
                            ts.tensor, ts.offset,
                            [ts.ap[0], [4, nreg], [1, 4], [0, 2]])
                        ex_out = bass.AP(
                            gts.tensor, gts.offset + 128,
                            [gts.ap[0], [136, nreg], [1, 8]])
                        nc.scalar.activation(ex_out, ts_in, AF.Exp)
                    # phase 2: msg = h * ex (in0 fp8 h, in1 bf16 ex dup, out
                    # bf16); one head per region on gpsimd to balance DVE
                    for (gt, nreg, adoff, r, gts) in regions:
                        if nreg == 0:
                            continue
                        for h in range(4):
                            g4o = bass.AP(
                                gts.tensor, gts.offset + h * 32,
                                [gts.ap[0], [136, nreg], [2, 16], [1, 2]])
                            g4i = bass.AP(
                                gt.tensor, gt.offset + h * 32,
                                [gt.ap[0], [136, nreg], [2, 16], [1, 2]])
                            exb = bass.AP(
                                gts.tensor, gts.offset + 128 + h * 2,
                                [gts.ap[0], [136, nreg], [0, 16], [1, 2]])
                            eng = (nc.gpsimd if (h >= 3 or
                                   (h == 2 and r == 1)) else nc.vector)
                            eng.tensor_tensor(g4o, g4i, exb, OP.mult)

                    # ---- per-slot accumulate + finalize ----
                    for s in slots:
                        chunks = slot_chunks[s]
                        acc = egacc.tile([128, 136], f32, tag="acc")
                        first_full = chunks[0][4] == 128
                        if not first_full:
                            nc.tensor.matmul(acc[:], orow_t[:], zrow_t[:],
                                             start=True, stop=False,
                                             skip_group_check=True)
                        for i, (r, ci, wc, wo, wl) in enumerate(chunks):
                            gts = regions[r][4]
                            nc.tensor.matmul(
                                acc[wo:wo + wl, :],
                                inds[r][:, wc:wc + wl],
                                gts[:, ci, 0:136],
                                start=(i == 0 and first_full),
                                stop=(i == len(chunks) - 1),
                                skip_group_check=True)

                        # normalize + bias + ELU (node-major)
                        dinv = egs.tile([128, 4], f32, tag="dinv")
                        den = bass.AP(acc.tensor, acc.offset + 128,
                                      [acc.ap[0], [2, 4]])
                        nc.vector.reciprocal(dinv[:], den)
                        gat = egs.tile([128, 128], f32, tag="gat")
                        dinvb = bass.AP(dinv.tensor, dinv.offset,
                                        [dinv.ap[0], [1, 4], [0, 32]])
                        nc.vector.tensor_tensor(gat[:], acc[:, 0:128], dinvb,
                                                OP.mult)
                        # transpose; GAT bias becomes per-partition Act bias,
                        # ELU = relu(x) - relu(1 - exp(x)) folds it in.
                        sub = s % 4
                        if sub == 0:
                            mgsz = min(4, TPC - s)
                            actT4 = msb.tile([128, 4 * NT], bf16, tag="actT4")
                        tp = egtp.tile([128, 128], f32, tag="tp2", bufs=1)
                        nc.tensor.transpose(tp[:], gat[:], eye_t[:])
                        t1 = egs.tile([128, 128], f32, tag="t1")
                        nc.scalar.activation(t1[:], tp[:], AF.Exp,
                                             bias=bias_col[:, 0:1])
                        nc.scalar.activation(t1[:], t1[:], AF.Relu,
                                             scale=-1.0, bias=1.0)
                        rl_t = egs.tile([128, 128], f32, tag="rl")
                        nc.scalar.activation(rl_t[:], tp[:], AF.Relu,
                                             bias=bias_col[:, 0:1])
                        nc.vector.tensor_sub(actT4[:, sub * NT:(sub + 1) * NT],
                                             rl_t[:], t1[:])

                        if sub == mgsz - 1:
                            g0 = s - sub
                            gn = mgsz * NT
                            a1 = msb.tile([128, 2, 512], bf16, tag="a1")
                            for j in range(2):
                                o1 = mps.tile([128, 512], f32, tag="o1")
                                nc.tensor.matmul(
                                    o1[:, 0:gn],
                                    W1_t[:, j * 128:(j + 1) * 128],
                                    actT4[:, 0:gn])
                                nc.scalar.activation(
                                    a1[:, j, 0:gn], o1[:, 0:gn], AF.Prelu,
                                    alpha=NEG_MLP, bias=b1_t[:, j:j + 1])
                            o2 = mps.tile([128, 512], f32, tag="o2")
                            for j in range(2):
                                nc.tensor.matmul(
                                    o2[:, 0:gn], W2_t[:, j, :],
                                    a1[:, j, 0:gn],
                                    start=(j == 0), stop=(j == 1))
                            a2 = msb.tile([128, 512], bf16, tag="a2")
                            nc.scalar.activation(
                                a2[:, 0:gn], o2[:, 0:gn], AF.Prelu,
                                alpha=NEG_MLP, bias=b2_t[:])
                            o3 = mps.tile([16, 512], f32, tag="sm", name="o3_t")
                            nc.tensor.matmul(o3[0:10, 0:gn], W3_t[:],
                                             a2[:, 0:gn])
                            z = msb.tile([16, 512], bf16, tag="z")
                            nc.scalar.activation(
                                z[0:10, 0:gn], o3[0:10, 0:gn], AF.Prelu,
                                alpha=NEG_MLP, bias=b3_t[0:10, :])
                            nc.scalar.activation(z[0:10, 0:gn], z[0:10, 0:gn],
                                                 AF.Exp)
                            ssum = mps.tile([16, 512], f32, tag="sm",
                                            name="ssum_t")[0:1, :]
                            nc.tensor.matmul(
                                ssum[:, 0:gn], ones_t[0:10, 0:1],
                                z[0:10, 0:gn])
                            sinv = msb.tile([1, 512], bf16, tag="sinv")
                            with nc.allow_low_precision(
                                    reason="softmax denom bf16"):
                                nc.vector.reciprocal(sinv[:, 0:gn],
                                                     ssum[:, 0:gn])
                            sx = mps.tile([16, 512], f32, tag="sm", name="sx_t")
                            nc.tensor.matmul(
                                sx[0:10, 0:gn], ones_t[0:1, 0:10],
                                sinv[:, 0:gn])
                            res = msb.tile([16, 512], f32, tag="res")
                            nc.vector.tensor_mul(
                                res[0:10, 0:gn], z[0:10, 0:gn], sx[0:10, 0:gn])
                            nc.sync.dma_start(
                                outT[:, g0 * NT:g0 * NT + gn], res[0:10, 0:gn])

    nc.compile()
    return nc


def _inputs_per_core(inputs, src_w, ad_w, dcol2, node_order, meta):
    x = np.asarray(inputs["x"], dtype=np.float32)
    fb = np.asarray(inputs["fb"], dtype=np.float32)
    Wg = np.asarray(inputs["Wg"], dtype=np.float32)
    bias_g = np.asarray(inputs["bias_g"], dtype=np.float32)
    att_src = np.asarray(inputs["att_src"], dtype=np.float32)
    att_dst = np.asarray(inputs["att_dst"], dtype=np.float32)
    W1 = np.asarray(inputs["W1"], dtype=np.float32)
    b1 = np.asarray(inputs["b1"], dtype=np.float32)
    W2 = np.asarray(inputs["W2"], dtype=np.float32)
    b2 = np.asarray(inputs["b2"], dtype=np.float32)
    W3 = np.asarray(inputs["W3"], dtype=np.float32)
    b3 = np.asarray(inputs["b3"], dtype=np.float32)

    x_pad = np.zeros((NPAD, NFP), dtype=np.float32)
    x_pad[:N, :NF] = x
    fb_pad = np.zeros((NFP, NMEL), dtype=np.float32)
    fb_pad[:NF] = fb

    att_blk_s = np.zeros((HC, 4), dtype=np.float32)
    att_blk_d = np.zeros((HC, 4), dtype=np.float32)
    for h in range(H):
        att_blk_s[h * C:(h + 1) * C, h] = att_src[h]
        att_blk_d[h * C:(h + 1) * C, h] = att_dst[h]

    b1p = np.zeros((128, 2), dtype=np.float32)
    b1p[:, 0] = b1[:128]
    b1p[:, 1] = b1[128:]
    b2p = b2.reshape(128, 1).astype(np.float32)
    b3p = np.zeros((128, 1), dtype=np.float32)
    b3p[:10, 0] = b3

    iota_f32 = np.tile(np.arange(128, dtype=np.float32)[None, :], (128, 1))
    common = {
        "fb_p": _to_fp8(fb_pad), "Wg": Wg, "Wg_bf": _to_bf16(Wg),
        "attb_s": att_blk_s, "attb_d": att_blk_d,
        "bias_col": bias_g.reshape(128, 1).astype(np.float32),
        "W1": _to_bf16(W1), "b1": b1p, "W2": _to_bf16(W2), "b2": b2p,
        "W3": _to_bf16(W3), "b3": b3p,
        "eye": np.eye(128, dtype=np.float32),
        "iota": _to_bf16(iota_f32),
        "ones": _to_bf16(np.ones((128, 16), dtype=np.float32)),
    }

    maps = []
    for k in range(NCORES):
        m = dict(common)
        m["xT_in"] = _to_fp8(
            np.ascontiguousarray(x_pad[node_order[k]].T))
        m["idx_src"] = src_w[k]
        m["idx_ad"] = ad_w[k]
        m["dcol2"] = _to_bf16(dcol2[k].reshape(128, -1))
        maps.append(m)
    return maps


def kernel(**inputs):
    from concourse.bass_utils import run_bass_kernel_spmd

    src_w, ad_w, dcol2, node_order, meta = _prep(inputs["edge_index"])
    key = ("nc", meta["TOTC"], tuple(meta["cpt"].reshape(-1)),
           tuple(meta["woff"]))
    if key not in _CACHE:
        _CACHE.clear()
        _CACHE[key] = _build(meta)
    nc = _CACHE[key]
    maps = _inputs_per_core(inputs, src_w, ad_w, dcol2, node_order, meta)
    res = run_bass_kernel_spmd(nc, maps, core_ids=list(range(NCORES)))
    out = np.zeros((NPAD, 10), dtype=np.float32)
    for k in range(NCORES):
        out[node_order[k]] = res.results[k]["outT"].T
    return out[:N]


# revision 71
# speedup vs baseline: 1.0197x; 1.0197x over previous
"""GAT (gnn_message_passing) Trainium2 Bass kernel — 8-core SPMD.

Contract: kernel(**inputs) -> np.ndarray with FULL inputs / FULL output.
Self-contained: hardcodes shapes; only imports the container's concourse stack.

Design:
- Stage A: per-core h = (x @ fb) @ Wg (x/fb in fp8) plus attention dots ->
  local 256B node rows [h fp8(128) | a_s bf16(4) | a_d bf16(4) | pad].
- AllGather (unmerged row-structured APs) shares the node table.
- Edge phase, grouped by GRP destination slots per gather batch, with the
  next group's gathers issued one group ahead (software pipelining):
  * 136B/row gathers of source rows (h + a_s), in two int16 index halves,
  * an 8B/row gather of per-edge destination attention (a_d),
  * batched indicator builds and ex = exp(prelu(a_s+a_d)) with duplicated
    bf16 pairs so DVE ops hit the 2x perf mode,
  * msg = h * ex (fp8 x bf16 -> bf16; one head per region on gpsimd),
  * per-chunk indicator matmuls accumulate numerators+denominators in PSUM,
  * normalize (Act scale-copies), ELU via transpose, fused MLP + softmax.
- Destination tiles are assigned to (core, slot) by balanced ranking so the
  SPMD max-padding across cores stays small.
"""
import sys

for _p in ("/opt/trn_rl_repo", "/root/.axon_site/_ro/trn_rl_repo"):
    if _p not in sys.path:
        sys.path.append(_p)

import numpy as np

# ---------------- problem constants (hardcoded per contract) ----------------
N = 50000
NF = 513
NFP = 640            # padded feature dim (5 * 128)
NMEL = 128
H, C = 4, 32
HC = H * C           # 128
E = 800000
NEG_ATT = 0.2
NEG_MLP = 0.01

NCORES = 8
TPC = 49             # tiles (slots) per core
NT = 128             # nodes per tile
NPC = TPC * NT       # 6272 nodes per core
NPAD = NCORES * NPC  # 50176
NTILES_G = NPAD // NT  # 392
RW = 256             # Hfull row elems (bf16 -> 512 B)
ADR = 128            # adrep row elems (bf16 -> 256 B row stride)
SPLIT = 32768        # max int16 gather index + 1
BASE_B = NPAD - SPLIT  # 17408; group-B gathers read Hfull[BASE_B:]
GRP = 6              # slots per gather group

_CACHE = {}


def _to_fp8(a):
    import ml_dtypes
    return np.asarray(a, dtype=np.float32).astype(ml_dtypes.float8_e4m3fn)


def _to_bf16(a):
    """f32 -> bf16 (round-to-nearest-even)."""
    try:
        import ml_dtypes
        return np.asarray(a, dtype=np.float32).astype(ml_dtypes.bfloat16)
    except ImportError:
        x = np.ascontiguousarray(a, dtype=np.float32).view(np.uint32)
        rounded = (((x >> 16) + ((x >> 15) & 1)) & 0xFFFF).astype(np.uint16)
        return rounded


def _prep(edge_index):
    """Host-side edge preprocessing. Returns per-core index/metadata arrays."""
    src = np.asarray(edge_index[0], dtype=np.int64)
    dst = np.asarray(edge_index[1], dtype=np.int64)
    loop = np.arange(N, dtype=np.int64)
    src = np.concatenate([src, loop])
    dst = np.concatenate([dst, loop])

    # ---- balanced tile -> (core, slot) assignment ----
    tile_of = dst // NT
    tcnt = np.bincount(tile_of, minlength=NTILES_G)
    order = np.argsort(-tcnt, kind="stable")      # rank r -> tile
    # slot r//8 gets ranks r with r%8 as core
    tile_core = np.empty(NTILES_G, dtype=np.int64)
    tile_slot = np.empty(NTILES_G, dtype=np.int64)
    for r, t in enumerate(order):
        tile_slot[t] = r // NCORES
        tile_core[t] = r % NCORES

    # node -> permuted global row
    nodes = np.arange(NPAD, dtype=np.int64)
    n_tile = nodes // NT
    perm_row = tile_core[n_tile] * NPC + tile_slot[n_tile] * NT + nodes % NT
    # node_order[k, local] = node id
    node_order = np.empty((NCORES, NPC), dtype=np.int64)
    node_order[perm_row // NPC, perm_row % NPC] = nodes

    # ---- per-edge core/slot/half ----
    ecore = tile_core[tile_of]
    eslot = tile_slot[tile_of]
    prow = perm_row[src]
    half = (prow >= SPLIT).astype(np.int64)
    srow = np.where(half == 1, prow - BASE_B, prow)
    dloc = dst % NT

    # sort per (core, slot, half, dst)
    so = np.lexsort((src, dloc, half, eslot, ecore))
    ecore, eslot, half, srow, dloc = (a[so] for a in (ecore, eslot, half, srow, dloc))

    # counts per (core, slot, half)
    cnt = np.zeros((NCORES, TPC, 2), dtype=np.int64)
    np.add.at(cnt, (ecore, eslot, half), 1)
    starts = np.zeros((NCORES, TPC, 2), dtype=np.int64)
    starts.reshape(-1)[1:] = np.cumsum(cnt.reshape(-1))[:-1]

    # chunks per (slot, half): max over cores
    cpt = np.maximum(1, -(-cnt.max(axis=0) // NT))   # [TPC, 2]

    ngrp = -(-TPC // GRP)
    # chunk order: per group g: A-chunks of slots g*4.., then B-chunks
    slot_chunk_off = np.zeros((TPC, 2), dtype=np.int64)   # abs chunk idx of each (s,half) run
    grp_a0 = np.zeros(ngrp, dtype=np.int64)
    grp_b0 = np.zeros(ngrp, dtype=np.int64)
    grp_end = np.zeros(ngrp, dtype=np.int64)
    co = 0
    for g in range(ngrp):
        slots = range(g * GRP, min((g + 1) * GRP, TPC))
        grp_a0[g] = co
        for s in slots:
            slot_chunk_off[s, 0] = co
            co += cpt[s, 0]
        grp_b0[g] = co
        for s in slots:
            slot_chunk_off[s, 1] = co
            co += cpt[s, 1]
        grp_end[g] = co
    TOTC = co
    TOTIDX = TOTC * NT

    # fill per-chunk per-core tables (chunk-major layout)
    src_rel = np.zeros((NCORES, TOTC, NT), dtype=np.int64)
    ad_idx = np.zeros((NCORES, TOTC, NT), dtype=np.int64)
    dloc_all = np.zeros((NCORES, TOTC, NT), dtype=np.int64)
    valid = np.zeros((NCORES, TOTC, NT), dtype=bool)
    for k in range(NCORES):
        for s in range(TPC):
            for hf in range(2):
                c0 = slot_chunk_off[s, hf]
                nch = int(cpt[s, hf])
                st, cn = starts[k, s, hf], int(cnt[k, s, hf])
                src_rel[k, c0:c0 + nch].reshape(-1)[:cn] = srow[st:st + cn]
                ad_idx[k, c0:c0 + nch].reshape(-1)[:cn] = (
                    eslot[st:st + cn] * NT + dloc[st:st + cn])
                dloc_all[k, c0:c0 + nch].reshape(-1)[:cn] = dloc[st:st + cn]
                valid[k, c0:c0 + nch].reshape(-1)[:cn] = True

    assert src_rel.min() >= 0 and src_rel.max() < SPLIT

    # windows per chunk (union over cores): narrowest of 32 at 32-aligned
    # offset <= 64, else 64 at {0, 64}, else full 128. PSUM matmul outputs
    # may start at partitions {0, 32, 64}.
    woff = np.zeros(TOTC, dtype=np.int64)
    wlen = np.full(TOTC, 128, dtype=np.int64)
    for c in range(TOTC):
        v = valid[:, c, :]
        if v.any():
            dl = dloc_all[:, c, :][v]
            lo, hi = int(dl.min()), int(dl.max())
            wo32 = min((lo // 32) * 32, 64)
            if hi < wo32 + 32:
                woff[c] = wo32
                wlen[c] = 32
            else:
                wo = 0 if lo < 64 else 64
                if hi < wo + 64:
                    woff[c] = wo
                    wlen[c] = 64

    # chunk -> owning slot (before region reorder)
    slot_of_chunk = np.zeros(TOTC, dtype=np.int64)
    for s in range(TPC):
        for hf in range(2):
            c0, nch = int(slot_chunk_off[s, hf]), int(cpt[s, hf])
            slot_of_chunk[c0:c0 + nch] = s

    # reorder chunks within each group REGION by wlen desc (stable) so one
    # batched indicator build covers each wl class; track per-slot lists.
    runs_grp = {}
    for g in range(ngrp):
        for r, (r0, r1) in enumerate(((int(grp_a0[g]), int(grp_b0[g])),
                                      (int(grp_b0[g]), int(grp_end[g])))):
            sl = slice(r0, r1)
            ro = np.argsort(-wlen[sl], kind="stable")
            for arr in (woff, wlen, slot_of_chunk):
                arr[sl] = arr[sl][ro]
            for arr in (src_rel, ad_idx, dloc_all, valid):
                arr[:, sl] = arr[:, sl][:, ro]
            rl = []
            i = r0
            while i < r1:
                w = int(wlen[i])
                j = i
                while j < r1 and int(wlen[j]) == w:
                    j += 1
                rl.append((w, i - r0, j - i))
                i = j
            runs_grp[(g, r)] = rl

    # per-slot chunk lists: (region, rel_idx, ind_col, woff, wlen)
    slot_chunks = {s: [] for s in range(TPC)}
    wcol = np.zeros(TOTC, dtype=np.int64)   # ind-arena col offset per chunk
    WSUM_A = WSUM_B = 0
    for g in range(ngrp):
        for r, (r0, r1) in enumerate(((int(grp_a0[g]), int(grp_b0[g])),
                                      (int(grp_b0[g]), int(grp_end[g])))):
            off = 0
            for c in range(r0, r1):
                wcol[c] = off
                off += int(wlen[c])
            if r == 0:
                WSUM_A = max(WSUM_A, off)
            else:
                WSUM_B = max(WSUM_B, off)
            for c in range(r0, r1):
                s = int(slot_of_chunk[c])
                slot_chunks[s].append((r, c - r0, int(wcol[c]), int(woff[c]),
                                       int(wlen[c])))
    for s in range(TPC):
        slot_chunks[s].sort(key=lambda t: -t[4])

    dst_rel = np.full((NCORES, TOTC, NT), 999.0, dtype=np.float32)
    for k in range(NCORES):
        dr = dloc_all[k] - woff[None, :].T
        dst_rel[k][valid[k]] = dr[valid[k]].astype(np.float32)

    # wrapped int16 index layout: [128, TOTIDX//16]
    def wrap(a):
        fl = a.reshape(NCORES, TOTIDX)
        w = fl.reshape(NCORES, TOTIDX // 16, 16).transpose(0, 2, 1)
        return np.tile(w, (1, 8, 1)).astype(np.int16)

    src_w = wrap(src_rel)
    ad_w = wrap(ad_idx)
    # dcol duplicated pairs, bf16: [NCORES, 128, TOTC, 2]
    dcol2 = np.repeat(dst_rel.transpose(0, 2, 1)[:, :, :, None], 2, axis=3)

    meta = {"cpt": cpt, "woff": woff, "wlen": wlen, "TOTC": TOTC,
            "TOTIDX": TOTIDX, "runs_grp": runs_grp, "wcol": wcol,
            "slot_chunks": slot_chunks, "slot_off": slot_chunk_off,
            "grp_a0": grp_a0, "grp_b0": grp_b0, "grp_end": grp_end,
            "ngrp": ngrp, "WSUM_A": WSUM_A, "WSUM_B": WSUM_B}
    return src_w, ad_w, dcol2, node_order, meta


def _build(meta):
    import concourse.bass as bass
    import concourse.bacc as bacc
    import concourse.mybir as mybir
    import concourse.tile as tile

    f32 = mybir.dt.float32
    bf16 = mybir.dt.bfloat16
    fp8 = mybir.dt.float8e4
    i16 = mybir.dt.int16
    AF = mybir.ActivationFunctionType
    OP = mybir.AluOpType

    cpt, woff, wlen = meta["cpt"], meta["woff"], meta["wlen"]
    TOTC, TOTIDX = meta["TOTC"], meta["TOTIDX"]
    runs_grp, slot_off = meta["runs_grp"], meta["slot_off"]
    slot_chunks, wcol = meta["slot_chunks"], meta["wcol"]
    grp_a0, grp_b0, grp_end = meta["grp_a0"], meta["grp_b0"], meta["grp_end"]
    ngrp = meta["ngrp"]

    GA_MAX = int(max(int(grp_b0[g] - grp_a0[g]) for g in range(ngrp)))
    WSUM_A, WSUM_B = meta["WSUM_A"], meta["WSUM_B"]
    GB_MAX = int(max(int(grp_end[g] - grp_b0[g]) for g in range(ngrp)))
    GT_MAX = int(max(int(grp_end[g] - grp_a0[g]) for g in range(ngrp)))
    TOT_MAX = int((cpt[:, 0] + cpt[:, 1]).max())

    nc = bacc.Bacc("TRN2", target_bir_lowering=False, debug=False)

    # ---- I/O ----
    xT_in = nc.dram_tensor("xT_in", [NFP, NPC], fp8, kind="ExternalInput")
    idx_src = nc.dram_tensor("idx_src", [128, TOTIDX // 16], i16, kind="ExternalInput")
    idx_ad = nc.dram_tensor("idx_ad", [128, TOTIDX // 16], i16, kind="ExternalInput")
    dcol_d = nc.dram_tensor("dcol2", [128, TOTC * 2], bf16, kind="ExternalInput")
    fb_p = nc.dram_tensor("fb_p", [NFP, NMEL], fp8, kind="ExternalInput")
    Wg_d = nc.dram_tensor("Wg", [NMEL, HC], f32, kind="ExternalInput")
    Wg_bf_d = nc.dram_tensor("Wg_bf", [NMEL, HC], bf16, kind="ExternalInput")
    attb_s = nc.dram_tensor("attb_s", [HC, 4], f32, kind="ExternalInput")
    attb_d = nc.dram_tensor("attb_d", [HC, 4], f32, kind="ExternalInput")
    bias_col_d = nc.dram_tensor("bias_col", [128, 1], f32, kind="ExternalInput")
    W1_d = nc.dram_tensor("W1", [HC, 256], bf16, kind="ExternalInput")
    b1_d = nc.dram_tensor("b1", [128, 2], f32, kind="ExternalInput")
    W2_d = nc.dram_tensor("W2", [256, HC], bf16, kind="ExternalInput")
    b2_d = nc.dram_tensor("b2", [128, 1], f32, kind="ExternalInput")
    W3_d = nc.dram_tensor("W3", [HC, 10], bf16, kind="ExternalInput")
    b3_d = nc.dram_tensor("b3", [128, 1], f32, kind="ExternalInput")
    eye_d = nc.dram_tensor("eye", [128, 128], f32, kind="ExternalInput")
    iota_d = nc.dram_tensor("iota", [128, 128], bf16, kind="ExternalInput")
    ones_d = nc.dram_tensor("ones", [128, 16], bf16, kind="ExternalInput")
    outT = nc.dram_tensor("outT", [10, NPC], f32, kind="ExternalOutput")

    core_ids = list(range(NCORES))

    def small_gather(g, out_ap, in_ap, idxs_ap, num_idxs, elem_size,
                     stride_b256):
        """Hand-rolled InstDMAGatherAnt allowing elem < 256B (stride stays a
        256B multiple, which is the actual ISA constraint)."""
        _in_ap = g.lower_ap_dma(in_ap, for_custom_bir_dma=True)
        return g.add_instruction(
            mybir.InstDMAGatherAnt(
                name=g.bass.get_next_instruction_name(),
                ins=[*_in_ap, g.lower_ap(idxs_ap),
                     g.lower_val_access(g.to_reg(num_idxs))],
                outs=[g.lower_ap(out_ap)],
                transpose=False,
                num_idxs=num_idxs,
                elem_size=elem_size,
                stride_bytes_256=stride_b256,
                gen_mode=0,
                single_packet=False,
                queue_num=0,
                sbuf_tokens_per_rank=0,
                sbuf_free_dim_per_rank=0,
                sbuf_free_dim_pad_per_rank=0,
                sbuf_byte_offset=0,
            ))

    with tile.TileContext(nc) as tc:
        with (
            tc.tile_pool(name="dram", bufs=1, space="DRAM") as dpool,
            tc.tile_pool(name="const", bufs=1) as cpool,
        ):
            Hext_loc = dpool.tile([NPC, RW], fp8)
            Hfull = dpool.tile([NPAD, RW], fp8, addr_space="Shared")

            # ---- constants to SBUF ----
            fb_t = cpool.tile([128, 5, NMEL], fp8)
            nc.sync.dma_start(fb_t[:], fb_p.rearrange("(b p) m -> p b m", p=128))
            Wg_t = cpool.tile([128, HC], f32)
            nc.sync.dma_start(Wg_t[:], Wg_d[:])
            Wg_bf = cpool.tile([128, HC], bf16)
            nc.sync.dma_start(Wg_bf[:], Wg_bf_d[:])
            atts_t = cpool.tile([128, 4], f32)
            nc.sync.dma_start(atts_t[:], attb_s[:])
            attd_t = cpool.tile([128, 4], f32)
            nc.sync.dma_start(attd_t[:], attb_d[:])
            bias_col = cpool.tile([128, 1], f32)
            nc.sync.dma_start(bias_col[:], bias_col_d[:])
            W1_t = cpool.tile([128, 256], bf16)
            nc.sync.dma_start(W1_t[:], W1_d[:])
            b1_t = cpool.tile([128, 2], f32)
            nc.sync.dma_start(b1_t[:], b1_d[:])
            W2_t = cpool.tile([128, 2, HC], bf16)
            nc.sync.dma_start(W2_t[:], W2_d.rearrange("(b p) m -> p b m", p=128))
            b2_t = cpool.tile([128, 1], f32)
            nc.sync.dma_start(b2_t[:], b2_d[:])
            W3_t = cpool.tile([128, 10], bf16)
            nc.sync.dma_start(W3_t[:], W3_d[:])
            b3_t = cpool.tile([128, 1], f32)
            nc.sync.dma_start(b3_t[:], b3_d[:])
            eye_t = cpool.tile([128, 128], f32)
            nc.sync.dma_start(eye_t[:], eye_d[:])
            zrow_t = cpool.tile([1, 136], bf16)
            nc.vector.memset(zrow_t[:], 0.0)
            orow_t = cpool.tile([1, 128], bf16)
            nc.vector.memset(orow_t[:], 1.0)
            iota_t = cpool.tile([128, 128], bf16)
            nc.sync.dma_start(iota_t[:], iota_d[:])
            ones_t = cpool.tile([128, 16], bf16)
            nc.sync.dma_start(ones_t[:], ones_d[:])
            dcol_t = cpool.tile([128, TOTC * 2], bf16)
            nc.scalar.dma_start(dcol_t[:], dcol_d[:])

            # WgT, Wgatt (h -> [h | a_s | a_d] projection columns)
            WgT_t = cpool.tile([128, 128], f32)
            WgA_t = cpool.tile([128, HC + 8], bf16)
            with tc.tile_pool(name="cpsum", bufs=1, space="PSUM") as cpsum:
                WgT_ps = cpsum.tile([128, 128], f32)
                nc.tensor.transpose(WgT_ps[:], Wg_t[:], eye_t[:])
                nc.vector.tensor_copy(WgT_t[:], WgT_ps[:])
                Wgatt_ps = cpsum.tile([128, 8], f32)
                nc.tensor.matmul(Wgatt_ps[:, 0:4], WgT_t[:], atts_t[:])
                nc.tensor.matmul(Wgatt_ps[:, 4:8], WgT_t[:], attd_t[:])
                nc.vector.tensor_copy(WgA_t[:, 0:HC], Wg_bf[:])
                nc.vector.tensor_copy(WgA_t[:, HC:HC + 8], Wgatt_ps[:])

            # ================= stage A =================
            with (
                tc.tile_pool(name="sa_sb", bufs=4) as sa,
                tc.tile_pool(name="sa_ps", bufs=3, space="PSUM") as saps,
                tc.tile_pool(name="sa_ps1", bufs=4, space="PSUM") as saps1,
            ):
                xT8 = None
                for g0 in range(0, TPC, 4):
                    gsz = min(4, TPC - g0)
                    gn = gsz * NT
                    if g0 % 8 == 0:
                        x8n = min(8, TPC - g0) * NT
                        xT8 = sa.tile([128, 5, 8 * NT], fp8, tag="xT", bufs=2)
                        nc.sync.dma_start(
                            xT8[:, :, 0:x8n],
                            xT_in.rearrange("(b p) n -> p b n", p=128)[
                                :, :, g0 * NT:g0 * NT + x8n])
                        xoff = 0
                    else:
                        xoff = 4 * NT
                    h1T_ps = saps.tile([128, 4 * NT], f32, tag="h1T")
                    for b in range(5):
                        nc.tensor.matmul(
                            h1T_ps[:, 0:gn],
                            fb_t[:, b, :],
                            xT8[:, b, xoff:xoff + gn],
                            start=(b == 0), stop=(b == 4))
                    h1T = sa.tile([128, 4 * NT], bf16, tag="h1Ts")
                    nc.vector.tensor_copy(h1T[:, 0:gn], h1T_ps[:, 0:gn])
                    hrow4 = sa.tile([128, 4, RW], fp8, tag="hrow4")
                    for u in range(gsz):
                        h_ps = saps1.tile([128, HC + 8], f32, tag="hps")
                        lhs = h1T[:, u * NT:(u + 1) * NT]
                        nc.tensor.matmul(h_ps[:, 0:HC + 8], lhs, WgA_t[:])
                        nc.scalar.activation(hrow4[:, u, 0:HC],
                                             h_ps[:, 0:HC], AF.Copy)
                        nc.scalar.activation(
                            hrow4[:, u, HC:HC + 16].bitcast(bf16),
                            h_ps[:, HC:HC + 8], AF.Copy)
                    nc.sync.dma_start(
                        Hext_loc[g0 * NT:g0 * NT + gn, 0:HC + 16].rearrange(
                            "(u p) c -> p u c", p=128),
                        hrow4[:, 0:gsz, 0:HC + 16])

                # AllGather with unmerged (opt=False) row-major APs.
                g = nc.gpsimd
                g.add_instruction(
                    mybir.InstCollectiveCompute(
                        name=f"I-{g.bass.next_id()}",
                        kind="AllGather",
                        op=mybir.AluOpType.bypass,
                        replica_groups=[core_ids],
                        ins=[g.lower_ap(Hext_loc[:], opt=False)],
                        outs=[g.lower_ap(Hfull[:], opt=False)],
                        unique_tensors="No",
                        cc_dim="Partition",
                    ))

            # ================= edge phase + MLP =================
            with (
                tc.tile_pool(name="eg_g", bufs=2) as egg,
                tc.tile_pool(name="eg_sb", bufs=3) as egs,
                tc.tile_pool(name="eg_ind", bufs=2) as egi,
                tc.tile_pool(name="eg_acc", bufs=3, space="PSUM") as egacc,
                tc.tile_pool(name="eg_tp", bufs=2, space="PSUM") as egtp,
                tc.tile_pool(name="mlp_sb", bufs=2) as msb,
                tc.tile_pool(name="mlp_ps", bufs=1, space="PSUM") as mps,
            ):
                actT4 = None
                mgsz = 4

                def issue_gathers(gidx):
                    a0, b0, e0 = (int(grp_a0[gidx]), int(grp_b0[gidx]),
                                  int(grp_end[gidx]))
                    nA, nB, nT_ = b0 - a0, e0 - b0, e0 - a0
                    isg = egg.tile([128, GT_MAX * 8], i16, tag="isg", bufs=3,
                                   name=f"isg_{gidx}")
                    nc.scalar.dma_start(isg[:, 0:nT_ * 8],
                                        idx_src[:, a0 * 8:e0 * 8])
                    adg = egg.tile([128, GT_MAX * 8], i16, tag="adg", bufs=3,
                                   name=f"adg_{gidx}")
                    nc.scalar.dma_start(adg[:, 0:nT_ * 8],
                                        idx_ad[:, a0 * 8:e0 * 8])
                    gAt = egg.tile([128, GA_MAX, HC + 8], fp8, tag="gA",
                                   bufs=3, name=f"gA_{gidx}")
                    small_gather(nc.gpsimd, gAt[:, 0:nA, :],
                                 Hfull[:, 0:HC + 8],
                                 isg[:, 0:nA * 8],
                                 num_idxs=nA * NT, elem_size=HC + 8,
                                 stride_b256=1)
                    adt = egg.tile([128, GT_MAX, 8], fp8, tag="adt", bufs=3,
                                   name=f"adt_{gidx}")
                    small_gather(nc.gpsimd, adt[:, 0:nT_, :],
                                 Hext_loc[:, HC + 8:HC + 16],
                                 adg[:, 0:nT_ * 8],
                                 num_idxs=nT_ * NT, elem_size=8, stride_b256=1)
                    gBt = egg.tile([128, GB_MAX, HC + 8], fp8, tag="gB",
                                   bufs=3, name=f"gB_{gidx}")
                    small_gather(nc.gpsimd, gBt[:, 0:nB, :],
                                 Hfull[BASE_B:NPAD, 0:HC + 8],
                                 isg[:, nA * 8:nT_ * 8],
                                 num_idxs=nB * NT, elem_size=HC + 8,
                                 stride_b256=1)
                    return gAt, gBt, adt

                pend = issue_gathers(0)
                for gidx in range(ngrp):
                    s0 = gidx * GRP
                    slots = list(range(s0, min(s0 + GRP, TPC)))
                    a0, b0, e0 = (int(grp_a0[gidx]), int(grp_b0[gidx]),
                                  int(grp_end[gidx]))
                    nA, nB, nT_ = b0 - a0, e0 - b0, e0 - a0
                    gAt, gBt, adt = pend
                    if gidx + 1 < ngrp:
                        pend = issue_gathers(gidx + 1)

                    # ---- per-region batched attention prep ----
                    gsA = egg.tile([128, GA_MAX, 136], bf16, tag="gsA",
                                   bufs=3, name=f"gsA_{gidx}")
                    gsB = egg.tile([128, GB_MAX, 136], bf16, tag="gsB",
                                   bufs=2, name=f"gsB_{gidx}")
                    regions = [(gAt, nA, 0, 0, gsA), (gBt, nB, b0 - a0, 1, gsB)]
                    inds = []
                    # phase 1: indicator builds (constants only) + a_s+a_d,
                    # prelu, exp for both regions
                    for (gt, nreg, adoff, r, gts) in regions:
                        ind = egi.tile(
                            [128, WSUM_A if r == 0 else WSUM_B], bf16,
                            tag="indA" if r == 0 else "indB",
                            name=f"ind_{gidx}_{r}")
                        inds.append(ind)
                        if nreg == 0:
                            continue
                        roff = a0 if r == 0 else b0
                        for (wl, r0, rl) in runs_grp[(gidx, r)]:
                            i0 = bass.AP(
                                iota_t.tensor, iota_t.offset,
                                [iota_t.ap[0], [0, rl], [2, wl // 2], [1, 2]])
                            i1 = bass.AP(
                                dcol_t.tensor,
                                dcol_t.offset + (roff + r0) * 2,
                                [dcol_t.ap[0], [2, rl], [0, wl // 2], [1, 2]])
                            o0 = bass.AP(
                                ind.tensor,
                                ind.offset + int(wcol[roff + r0]),
                                [ind.ap[0], [wl, rl], [2, wl // 2], [1, 2]])
                            nc.vector.tensor_tensor(o0, i0, i1, OP.is_equal)
                        # t = a_s + a_d (bf16 views of fp8 tiles)
                        ts = egs.tile([128, GA_MAX, 4], bf16, tag="ts",
                                      name=f"ts_{gidx}_{r}")
                        nc.vector.tensor_tensor(
                            ts[:, 0:nreg, :],
                            gt[:, 0:nreg, 128:136].bitcast(bf16),
                            adt[:, adoff:adoff + nreg, :].bitcast(bf16),
                            OP.add)
                        nc.scalar.activation(ts[:, 0:nreg, :], ts[:, 0:nreg, :],
                                             AF.Prelu, alpha=NEG_ATT)
                        # exp, duplicated pairs -> gts cols 128:136 (bf16)
                        ts_in = bass.AP(
                            ts.tensor, ts.offset,
                            [ts.ap[0], [4, nreg], [1, 4], [0, 2]])
                        ex_out = bass.AP(
                            gts.tensor, gts.offset + 128,
                            [gts.ap[0], [136, nreg], [1, 8]])
                        nc.scalar.activation(ex_out, ts_in, AF.Exp)
                    # phase 2: msg = h * ex (in0 fp8 h, in1 bf16 ex dup, out
                    # bf16); one head per region on gpsimd to balance DVE
                    for (gt, nreg, adoff, r, gts) in regions:
                        if nreg == 0:
                            continue
                        for h in range(4):
                            g4o = bass.AP(
                                gts.tensor, gts.offset + h * 32,
                                [gts.ap[0], [136, nreg], [2, 16], [1, 2]])
                            g4i = bass.AP(
                                gt.tensor, gt.offset + h * 32,
                                [gt.ap[0], [136, nreg], [2, 16], [1, 2]])
                            exb = bass.AP(
                                gts.tensor, gts.offset + 128 + h * 2,
                                [gts.ap[0], [136, nreg], [0, 16], [1, 2]])
                            eng = (nc.gpsimd if (h >= 3 or
                                   (h == 2 and r == 1)) else nc.vector)
                            eng.tensor_tensor(g4o, g4i, exb, OP.mult)

                    # ---- per-slot accumulate + finalize ----
                    for s in slots:
                        chunks = slot_chunks[s]
                        acc = egacc.tile([128, 136], f32, tag="acc")
                        first_full = chunks[0][4] == 128
                        if not first_full:
                            nc.tensor.matmul(acc[:], orow_t[:], zrow_t[:],
                                             start=True, stop=False,
                                             skip_group_check=True)
                        for i, (r, ci, wc, wo, wl) in enumerate(chunks):
                            gts = regions[r][4]
                            nc.tensor.matmul(
                                acc[wo:wo + wl, :],
                                inds[r][:, wc:wc + wl],
                                gts[:, ci, 0:136],
                                start=(i == 0 and first_full),
                                stop=(i == len(chunks) - 1),
                                skip_group_check=True)

                        # normalize + bias + ELU (node-major)
                        dinv = egs.tile([128, 4], f32, tag="dinv")
                        den = bass.AP(acc.tensor, acc.offset + 128,
                                      [acc.ap[0], [2, 4]])
                        nc.vector.reciprocal(dinv[:], den)
                        gat = egs.tile([128, 128], f32, tag="gat")
                        dinvb = bass.AP(dinv.tensor, dinv.offset,
                                        [dinv.ap[0], [1, 4], [0, 32]])
                        nc.vector.tensor_tensor(gat[:], acc[:, 0:128], dinvb,
                                                OP.mult)
                        # transpose; GAT bias becomes per-partition Act bias,
                        # ELU = relu(x) - relu(1 - exp(x)) folds it in.
                        sub = s % 4
                        if sub == 0:
                            mgsz = min(4, TPC - s)
                            actT4 = msb.tile([128, 4 * NT], bf16, tag="actT4")
                        tp = egtp.tile([128, 128], f32, tag="tp2", bufs=1)
                        nc.tensor.transpose(tp[:], gat[:], eye_t[:])
                        t1 = egs.tile([128, 128], f32, tag="t1")
                        nc.scalar.activation(t1[:], tp[:], AF.Exp,
                                             bias=bias_col[:, 0:1])
                        nc.scalar.activation(t1[:], t1[:], AF.Relu,
                                             scale=-1.0, bias=1.0)
                        rl_t = egs.tile([128, 128], f32, tag="rl")
                        nc.scalar.activation(rl_t[:], tp[:], AF.Relu,
                                             bias=bias_col[:, 0:1])
                        nc.vector.tensor_sub(actT4[:, sub * NT:(sub + 1) * NT],
                                             rl_t[:], t1[:])

                        if sub == mgsz - 1:
                            g0 = s - sub
                            gn = mgsz * NT
                            a1 = msb.tile([128, 2, 512], bf16, tag="a1")
                            for j in range(2):
                                o1 = mps.tile([128, 512], f32, tag="o1")
                                nc.tensor.matmul(
                                    o1[:, 0:gn],
                                    W1_t[:, j * 128:(j + 1) * 128],
                                    actT4[:, 0:gn])
                                nc.scalar.activation(
                                    a1[:, j, 0:gn], o1[:, 0:gn], AF.Prelu,
                                    alpha=NEG_MLP, bias=b1_t[:, j:j + 1])
                            o2 = mps.tile([128, 512], f32, tag="o2")
                            for j in range(2):
                                nc.tensor.matmul(
                                    o2[:, 0:gn], W2_t[:, j, :],
                                    a1[:, j, 0:gn],
                                    start=(j == 0), stop=(j == 1))
                            a2 = msb.tile([128, 512], bf16, tag="a2")
                            nc.scalar.activation(
                                a2[:, 0:gn], o2[:, 0:gn], AF.Prelu,
                                alpha=NEG_MLP, bias=b2_t[:])
                            o3 = mps.tile([16, 512], f32, tag="sm", name="o3_t")
                            nc.tensor.matmul(o3[0:10, 0:gn], W3_t[:],
                                             a2[:, 0:gn])
                            z = msb.tile([16, 512], bf16, tag="z")
                            nc.scalar.activation(
                                z[0:10, 0:gn], o3[0:10, 0:gn], AF.Prelu,
                                alpha=NEG_MLP, bias=b3_t[0:10, :])
                            nc.scalar.activation(z[0:10, 0:gn], z[0:10, 0:gn],
                                                 AF.Exp)
                            ssum = mps.tile([16, 512], f32, tag="sm",
                                            name="ssum_t")[0:1, :]
                            nc.tensor.matmul(
                                ssum[:, 0:gn], ones_t[0:10, 0:1],
                                z[0:10, 0:gn])
                            sinv = msb.tile([1, 512], bf16, tag="sinv")
                            with nc.allow_low_precision(
                                    reason="softmax denom bf16"):
                                nc.vector.reciprocal(sinv[:, 0:gn],
                                                     ssum[:, 0:gn])
                            sx = mps.tile([16, 512], f32, tag="sm", name="sx_t")
                            nc.tensor.matmul(
                                sx[0:10, 0:gn], ones_t[0:1, 0:10],
                                sinv[:, 0:gn])
                            res = msb.tile([16, 512], f32, tag="res")
                            nc.vector.tensor_mul(
                                res[0:10, 0:gn], z[0:10, 0:gn], sx[0:10, 0:gn])
                            nc.sync.dma_start(
                                outT[:, g0 * NT:g0 * NT + gn], res[0:10, 0:gn])

    nc.compile()
    return nc


def _inputs_per_core(inputs, src_w, ad_w, dcol2, node_order, meta):
    x = np.asarray(inputs["x"], dtype=np.float32)
    fb = np.asarray(inputs["fb"], dtype=np.float32)
    Wg = np.asarray(inputs["Wg"], dtype=np.float32)
    bias_g = np.asarray(inputs["bias_g"], dtype=np.float32)
    att_src = np.asarray(inputs["att_src"], dtype=np.float32)
    att_dst = np.asarray(inputs["att_dst"], dtype=np.float32)
    W1 = np.asarray(inputs["W1"], dtype=np.float32)
    b1 = np.asarray(inputs["b1"], dtype=np.float32)
    W2 = np.asarray(inputs["W2"], dtype=np.float32)
    b2 = np.asarray(inputs["b2"], dtype=np.float32)
    W3 = np.asarray(inputs["W3"], dtype=np.float32)
    b3 = np.asarray(inputs["b3"], dtype=np.float32)

    x_pad = np.zeros((NPAD, NFP), dtype=np.float32)
    x_pad[:N, :NF] = x
    fb_pad = np.zeros((NFP, NMEL), dtype=np.float32)
    fb_pad[:NF] = fb

    att_blk_s = np.zeros((HC, 4), dtype=np.float32)
    att_blk_d = np.zeros((HC, 4), dtype=np.float32)
    for h in range(H):
        att_blk_s[h * C:(h + 1) * C, h] = att_src[h]
        att_blk_d[h * C:(h + 1) * C, h] = att_dst[h]

    b1p = np.zeros((128, 2), dtype=np.float32)
    b1p[:, 0] = b1[:128]
    b1p[:, 1] = b1[128:]
    b2p = b2.reshape(128, 1).astype(np.float32)
    b3p = np.zeros((128, 1), dtype=np.float32)
    b3p[:10, 0] = b3

    iota_f32 = np.tile(np.arange(128, dtype=np.float32)[None, :], (128, 1))
    common = {
        "fb_p": _to_fp8(fb_pad), "Wg": Wg, "Wg_bf": _to_bf16(Wg),
        "attb_s": att_blk_s, "attb_d": att_blk_d,
        "bias_col": bias_g.reshape(128, 1).astype(np.float32),
        "W1": _to_bf16(W1), "b1": b1p, "W2": _to_bf16(W2), "b2": b2p,
        "W3": _to_bf16(W3), "b3": b3p,
        "eye": np.eye(128, dtype=np.float32),
        "iota": _to_bf16(iota_f32),
        "ones": _to_bf16(np.ones((128, 16), dtype=np.float32)),
    }

    maps = []
    for k in range(NCORES):
        m = dict(common)
        m["xT_in"] = _to_fp8(
            np.ascontiguousarray(x_pad[node_order[k]].T))
        m["idx_src"] = src_w[k]
        m["idx_ad"] = ad_w[k]
        m["dcol2"] = _to_bf16(dcol2[k].reshape(128, -1))
        maps.append(m)
    return maps


def kernel(**inputs):
    from concourse.bass_utils import run_bass_kernel_spmd

    src_w, ad_w, dcol2, node_order, meta = _prep(inputs["edge_index"])
    key = ("nc", meta["TOTC"], tuple(meta["cpt"].reshape(-1)),
           tuple(meta["woff"]))
    if key not in _CACHE:
        _CACHE.clear()
        _CACHE[key] = _build(meta)
    nc = _CACHE[key]
    maps = _inputs_per_core(inputs, src_w, ad_w, dcol2, node_order, meta)
    res = run_bass_kernel_spmd(nc, maps, core_ids=list(range(NCORES)))
    out = np.zeros((NPAD, 10), dtype=np.float32)
    for k in range(NCORES):
        out[node_order[k]] = res.results[k]["outT"].T
    return out[:N]


# revision 72
# speedup vs baseline: 1.0288x; 1.0090x over previous
"""GAT (gnn_message_passing) Trainium2 Bass kernel — 8-core SPMD.

Contract: kernel(**inputs) -> np.ndarray with FULL inputs / FULL output.
Self-contained: hardcodes shapes; only imports the container's concourse stack.

Design:
- Stage A: per-core h = (x @ fb) @ Wg (x/fb in fp8) plus attention dots ->
  local 256B node rows [h fp8(128) | a_s bf16(4) | a_d bf16(4) | pad].
- AllGather (unmerged row-structured APs) shares the node table.
- Edge phase, grouped by GRP destination slots per gather batch, with the
  next group's gathers issued one group ahead (software pipelining):
  * 136B/row gathers of source rows (h + a_s), in two int16 index halves,
  * an 8B/row gather of per-edge destination attention (a_d),
  * batched indicator builds and ex = exp(prelu(a_s+a_d)) with duplicated
    bf16 pairs so DVE ops hit the 2x perf mode,
  * msg = h * ex (fp8 x bf16 -> bf16; one head per region on gpsimd),
  * per-chunk indicator matmuls accumulate numerators+denominators in PSUM,
  * normalize (Act scale-copies), ELU via transpose, fused MLP + softmax.
- Destination tiles are assigned to (core, slot) by balanced ranking so the
  SPMD max-padding across cores stays small.
"""
import sys

for _p in ("/opt/trn_rl_repo", "/root/.axon_site/_ro/trn_rl_repo"):
    if _p not in sys.path:
        sys.path.append(_p)

import numpy as np

# ---------------- problem constants (hardcoded per contract) ----------------
N = 50000
NF = 513
NFP = 640            # padded feature dim (5 * 128)
NMEL = 128
H, C = 4, 32
HC = H * C           # 128
E = 800000
NEG_ATT = 0.2
NEG_MLP = 0.01

NCORES = 8
TPC = 49             # tiles (slots) per core
NT = 128             # nodes per tile
NPC = TPC * NT       # 6272 nodes per core
NPAD = NCORES * NPC  # 50176
NTILES_G = NPAD // NT  # 392
RW = 256             # Hfull row elems (bf16 -> 512 B)
ADR = 128            # adrep row elems (bf16 -> 256 B row stride)
SPLIT = 32768        # max int16 gather index + 1
BASE_B = NPAD - SPLIT  # 17408; group-B gathers read Hfull[BASE_B:]
GRP = 6              # slots per gather group

_CACHE = {}


def _to_fp8(a):
    import ml_dtypes
    return np.asarray(a, dtype=np.float32).astype(ml_dtypes.float8_e4m3fn)


def _to_bf16(a):
    """f32 -> bf16 (round-to-nearest-even)."""
    try:
        import ml_dtypes
        return np.asarray(a, dtype=np.float32).astype(ml_dtypes.bfloat16)
    except ImportError:
        x = np.ascontiguousarray(a, dtype=np.float32).view(np.uint32)
        rounded = (((x >> 16) + ((x >> 15) & 1)) & 0xFFFF).astype(np.uint16)
        return rounded


def _prep(edge_index):
    """Host-side edge preprocessing. Returns per-core index/metadata arrays."""
    src = np.asarray(edge_index[0], dtype=np.int64)
    dst = np.asarray(edge_index[1], dtype=np.int64)
    loop = np.arange(N, dtype=np.int64)
    src = np.concatenate([src, loop])
    dst = np.concatenate([dst, loop])

    # ---- balanced tile -> (core, slot) assignment ----
    tile_of = dst // NT
    tcnt = np.bincount(tile_of, minlength=NTILES_G)
    order = np.argsort(-tcnt, kind="stable")      # rank r -> tile
    # slot r//8 gets ranks r with r%8 as core
    tile_core = np.empty(NTILES_G, dtype=np.int64)
    tile_slot = np.empty(NTILES_G, dtype=np.int64)
    for r, t in enumerate(order):
        tile_slot[t] = r // NCORES
        tile_core[t] = r % NCORES

    # node -> permuted global row
    nodes = np.arange(NPAD, dtype=np.int64)
    n_tile = nodes // NT
    perm_row = tile_core[n_tile] * NPC + tile_slot[n_tile] * NT + nodes % NT
    # node_order[k, local] = node id
    node_order = np.empty((NCORES, NPC), dtype=np.int64)
    node_order[perm_row // NPC, perm_row % NPC] = nodes

    # ---- per-edge core/slot/half ----
    ecore = tile_core[tile_of]
    eslot = tile_slot[tile_of]
    prow = perm_row[src]
    half = (prow >= SPLIT).astype(np.int64)
    srow = np.where(half == 1, prow - BASE_B, prow)
    dloc = dst % NT

    # sort per (core, slot, half, dst)
    so = np.lexsort((src, dloc, half, eslot, ecore))
    ecore, eslot, half, srow, dloc = (a[so] for a in (ecore, eslot, half, srow, dloc))

    # counts per (core, slot, half)
    cnt = np.zeros((NCORES, TPC, 2), dtype=np.int64)
    np.add.at(cnt, (ecore, eslot, half), 1)
    starts = np.zeros((NCORES, TPC, 2), dtype=np.int64)
    starts.reshape(-1)[1:] = np.cumsum(cnt.reshape(-1))[:-1]

    # chunks per (slot, half): max over cores
    cpt = np.maximum(1, -(-cnt.max(axis=0) // NT))   # [TPC, 2]

    ngrp = -(-TPC // GRP)
    # chunk order: per group g: A-chunks of slots g*4.., then B-chunks
    slot_chunk_off = np.zeros((TPC, 2), dtype=np.int64)   # abs chunk idx of each (s,half) run
    grp_a0 = np.zeros(ngrp, dtype=np.int64)
    grp_b0 = np.zeros(ngrp, dtype=np.int64)
    grp_end = np.zeros(ngrp, dtype=np.int64)
    co = 0
    for g in range(ngrp):
        slots = range(g * GRP, min((g + 1) * GRP, TPC))
        grp_a0[g] = co
        for s in slots:
            slot_chunk_off[s, 0] = co
            co += cpt[s, 0]
        grp_b0[g] = co
        for s in slots:
            slot_chunk_off[s, 1] = co
            co += cpt[s, 1]
        grp_end[g] = co
    TOTC = co
    TOTIDX = TOTC * NT

    # fill per-chunk per-core tables (chunk-major layout)
    src_rel = np.zeros((NCORES, TOTC, NT), dtype=np.int64)
    ad_idx = np.zeros((NCORES, TOTC, NT), dtype=np.int64)
    dloc_all = np.zeros((NCORES, TOTC, NT), dtype=np.int64)
    valid = np.zeros((NCORES, TOTC, NT), dtype=bool)
    for k in range(NCORES):
        for s in range(TPC):
            for hf in range(2):
                c0 = slot_chunk_off[s, hf]
                nch = int(cpt[s, hf])
                st, cn = starts[k, s, hf], int(cnt[k, s, hf])
                src_rel[k, c0:c0 + nch].reshape(-1)[:cn] = srow[st:st + cn]
                ad_idx[k, c0:c0 + nch].reshape(-1)[:cn] = (
                    eslot[st:st + cn] * NT + dloc[st:st + cn])
                dloc_all[k, c0:c0 + nch].reshape(-1)[:cn] = dloc[st:st + cn]
                valid[k, c0:c0 + nch].reshape(-1)[:cn] = True

    assert src_rel.min() >= 0 and src_rel.max() < SPLIT

    # windows per chunk (union over cores): narrowest of 32 at 32-aligned
    # offset <= 64, else 64 at {0, 64}, else full 128. PSUM matmul outputs
    # may start at partitions {0, 32, 64}.
    woff = np.zeros(TOTC, dtype=np.int64)
    wlen = np.full(TOTC, 128, dtype=np.int64)
    for c in range(TOTC):
        v = valid[:, c, :]
        if v.any():
            dl = dloc_all[:, c, :][v]
            lo, hi = int(dl.min()), int(dl.max())
            wo32 = min((lo // 32) * 32, 64)
            if hi < wo32 + 32:
                woff[c] = wo32
                wlen[c] = 32
            else:
                wo = 0 if lo < 64 else 64
                if hi < wo + 64:
                    woff[c] = wo
                    wlen[c] = 64

    # chunk -> owning slot (before region reorder)
    slot_of_chunk = np.zeros(TOTC, dtype=np.int64)
    for s in range(TPC):
        for hf in range(2):
            c0, nch = int(slot_chunk_off[s, hf]), int(cpt[s, hf])
            slot_of_chunk[c0:c0 + nch] = s

    # reorder chunks within each group REGION by wlen desc (stable) so one
    # batched indicator build covers each wl class; track per-slot lists.
    runs_grp = {}
    for g in range(ngrp):
        for r, (r0, r1) in enumerate(((int(grp_a0[g]), int(grp_b0[g])),
                                      (int(grp_b0[g]), int(grp_end[g])))):
            sl = slice(r0, r1)
            ro = np.argsort(-wlen[sl], kind="stable")
            for arr in (woff, wlen, slot_of_chunk):
                arr[sl] = arr[sl][ro]
            for arr in (src_rel, ad_idx, dloc_all, valid):
                arr[:, sl] = arr[:, sl][:, ro]
            rl = []
            i = r0
            while i < r1:
                w = int(wlen[i])
                j = i
                while j < r1 and int(wlen[j]) == w:
                    j += 1
                rl.append((w, i - r0, j - i))
                i = j
            runs_grp[(g, r)] = rl

    # per-slot chunk lists: (region, rel_idx, ind_col, woff, wlen)
    slot_chunks = {s: [] for s in range(TPC)}
    wcol = np.zeros(TOTC, dtype=np.int64)   # ind-arena col offset per chunk
    WSUM_A = WSUM_B = 0
    for g in range(ngrp):
        for r, (r0, r1) in enumerate(((int(grp_a0[g]), int(grp_b0[g])),
                                      (int(grp_b0[g]), int(grp_end[g])))):
            off = 0
            for c in range(r0, r1):
                wcol[c] = off
                off += int(wlen[c])
            if r == 0:
                WSUM_A = max(WSUM_A, off)
            else:
                WSUM_B = max(WSUM_B, off)
            for c in range(r0, r1):
                s = int(slot_of_chunk[c])
                slot_chunks[s].append((r, c - r0, int(wcol[c]), int(woff[c]),
                                       int(wlen[c])))
    for s in range(TPC):
        slot_chunks[s].sort(key=lambda t: -t[4])

    dst_rel = np.full((NCORES, TOTC, NT), 999.0, dtype=np.float32)
    for k in range(NCORES):
        dr = dloc_all[k] - woff[None, :].T
        dst_rel[k][valid[k]] = dr[valid[k]].astype(np.float32)

    # wrapped int16 index layout: [128, TOTIDX//16]
    def wrap(a):
        fl = a.reshape(NCORES, TOTIDX)
        w = fl.reshape(NCORES, TOTIDX // 16, 16).transpose(0, 2, 1)
        return np.tile(w, (1, 8, 1)).astype(np.int16)

    src_w = wrap(src_rel)
    ad_w = wrap(ad_idx)
    # dcol duplicated pairs, bf16: [NCORES, 128, TOTC, 2]
    dcol2 = np.repeat(dst_rel.transpose(0, 2, 1)[:, :, :, None], 2, axis=3)

    meta = {"cpt": cpt, "woff": woff, "wlen": wlen, "TOTC": TOTC,
            "TOTIDX": TOTIDX, "runs_grp": runs_grp, "wcol": wcol,
            "slot_chunks": slot_chunks, "slot_off": slot_chunk_off,
            "grp_a0": grp_a0, "grp_b0": grp_b0, "grp_end": grp_end,
            "ngrp": ngrp, "WSUM_A": WSUM_A, "WSUM_B": WSUM_B}
    return src_w, ad_w, dcol2, node_order, meta


def _build(meta):
    import concourse.bass as bass
    import concourse.bacc as bacc
    import concourse.mybir as mybir
    import concourse.tile as tile

    f32 = mybir.dt.float32
    bf16 = mybir.dt.bfloat16
    fp8 = mybir.dt.float8e4
    i16 = mybir.dt.int16
    AF = mybir.ActivationFunctionType
    OP = mybir.AluOpType

    cpt, woff, wlen = meta["cpt"], meta["woff"], meta["wlen"]
    TOTC, TOTIDX = meta["TOTC"], meta["TOTIDX"]
    runs_grp, slot_off = meta["runs_grp"], meta["slot_off"]
    slot_chunks, wcol = meta["slot_chunks"], meta["wcol"]
    grp_a0, grp_b0, grp_end = meta["grp_a0"], meta["grp_b0"], meta["grp_end"]
    ngrp = meta["ngrp"]

    GA_MAX = int(max(int(grp_b0[g] - grp_a0[g]) for g in range(ngrp)))
    WSUM_A, WSUM_B = meta["WSUM_A"], meta["WSUM_B"]
    GB_MAX = int(max(int(grp_end[g] - grp_b0[g]) for g in range(ngrp)))
    GT_MAX = int(max(int(grp_end[g] - grp_a0[g]) for g in range(ngrp)))
    TOT_MAX = int((cpt[:, 0] + cpt[:, 1]).max())

    nc = bacc.Bacc("TRN2", target_bir_lowering=False, debug=False)

    # ---- I/O ----
    xT_in = nc.dram_tensor("xT_in", [NFP, NPC], fp8, kind="ExternalInput")
    idx_src = nc.dram_tensor("idx_src", [128, TOTIDX // 16], i16, kind="ExternalInput")
    idx_ad = nc.dram_tensor("idx_ad", [128, TOTIDX // 16], i16, kind="ExternalInput")
    dcol_d = nc.dram_tensor("dcol2", [128, TOTC * 2], bf16, kind="ExternalInput")
    fb_p = nc.dram_tensor("fb_p", [NFP, NMEL], fp8, kind="ExternalInput")
    Wg_d = nc.dram_tensor("Wg", [NMEL, HC], f32, kind="ExternalInput")
    Wg_bf_d = nc.dram_tensor("Wg_bf", [NMEL, HC], bf16, kind="ExternalInput")
    attb_s = nc.dram_tensor("attb_s", [HC, 4], f32, kind="ExternalInput")
    attb_d = nc.dram_tensor("attb_d", [HC, 4], f32, kind="ExternalInput")
    bias_col_d = nc.dram_tensor("bias_col", [128, 1], f32, kind="ExternalInput")
    W1_d = nc.dram_tensor("W1", [HC, 256], bf16, kind="ExternalInput")
    b1_d = nc.dram_tensor("b1", [128, 2], f32, kind="ExternalInput")
    W2_d = nc.dram_tensor("W2", [256, HC], bf16, kind="ExternalInput")
    b2_d = nc.dram_tensor("b2", [128, 1], f32, kind="ExternalInput")
    W3_d = nc.dram_tensor("W3", [HC, 10], bf16, kind="ExternalInput")
    b3_d = nc.dram_tensor("b3", [128, 1], f32, kind="ExternalInput")
    eye_d = nc.dram_tensor("eye", [128, 128], f32, kind="ExternalInput")
    iota_d = nc.dram_tensor("iota", [128, 128], bf16, kind="ExternalInput")
    ones_d = nc.dram_tensor("ones", [128, 16], bf16, kind="ExternalInput")
    outT = nc.dram_tensor("outT", [10, NPC], f32, kind="ExternalOutput")

    core_ids = list(range(NCORES))

    def small_gather(g, out_ap, in_ap, idxs_ap, num_idxs, elem_size,
                     stride_b256):
        """Hand-rolled InstDMAGatherAnt allowing elem < 256B (stride stays a
        256B multiple, which is the actual ISA constraint)."""
        _in_ap = g.lower_ap_dma(in_ap, for_custom_bir_dma=True)
        return g.add_instruction(
            mybir.InstDMAGatherAnt(
                name=g.bass.get_next_instruction_name(),
                ins=[*_in_ap, g.lower_ap(idxs_ap),
                     g.lower_val_access(g.to_reg(num_idxs))],
                outs=[g.lower_ap(out_ap)],
                transpose=False,
                num_idxs=num_idxs,
                elem_size=elem_size,
                stride_bytes_256=stride_b256,
                gen_mode=0,
                single_packet=False,
                queue_num=0,
                sbuf_tokens_per_rank=0,
                sbuf_free_dim_per_rank=0,
                sbuf_free_dim_pad_per_rank=0,
                sbuf_byte_offset=0,
            ))

    with tile.TileContext(nc) as tc:
        with (
            tc.tile_pool(name="dram", bufs=1, space="DRAM") as dpool,
            tc.tile_pool(name="const", bufs=1) as cpool,
        ):
            Hext_loc = dpool.tile([NPC, RW], fp8)
            Hfull = dpool.tile([NPAD, RW], fp8, addr_space="Shared")

            # ---- constants to SBUF ----
            fb_t = cpool.tile([128, 5, NMEL], fp8)
            nc.sync.dma_start(fb_t[:], fb_p.rearrange("(b p) m -> p b m", p=128))
            Wg_t = cpool.tile([128, HC], f32)
            nc.sync.dma_start(Wg_t[:], Wg_d[:])
            Wg_bf = cpool.tile([128, HC], bf16)
            nc.sync.dma_start(Wg_bf[:], Wg_bf_d[:])
            atts_t = cpool.tile([128, 4], f32)
            nc.sync.dma_start(atts_t[:], attb_s[:])
            attd_t = cpool.tile([128, 4], f32)
            nc.sync.dma_start(attd_t[:], attb_d[:])
            bias_col = cpool.tile([128, 1], f32)
            nc.sync.dma_start(bias_col[:], bias_col_d[:])
            W1_t = cpool.tile([128, 256], bf16)
            nc.sync.dma_start(W1_t[:], W1_d[:])
            b1_t = cpool.tile([128, 2], f32)
            nc.sync.dma_start(b1_t[:], b1_d[:])
            W2_t = cpool.tile([128, 2, HC], bf16)
            nc.sync.dma_start(W2_t[:], W2_d.rearrange("(b p) m -> p b m", p=128))
            b2_t = cpool.tile([128, 1], f32)
            nc.sync.dma_start(b2_t[:], b2_d[:])
            W3_t = cpool.tile([128, 10], bf16)
            nc.sync.dma_start(W3_t[:], W3_d[:])
            b3_t = cpool.tile([128, 1], f32)
            nc.sync.dma_start(b3_t[:], b3_d[:])
            eye_t = cpool.tile([128, 128], f32)
            nc.sync.dma_start(eye_t[:], eye_d[:])
            zrow_t = cpool.tile([1, 136], bf16)
            nc.vector.memset(zrow_t[:], 0.0)
            orow_t = cpool.tile([1, 128], bf16)
            nc.vector.memset(orow_t[:], 1.0)
            iota_t = cpool.tile([128, 128], bf16)
            nc.sync.dma_start(iota_t[:], iota_d[:])
            ones_t = cpool.tile([128, 16], bf16)
            nc.sync.dma_start(ones_t[:], ones_d[:])
            dcol_t = cpool.tile([128, TOTC * 2], bf16)
            nc.scalar.dma_start(dcol_t[:], dcol_d[:])

            # WgT, Wgatt (h -> [h | a_s | a_d] projection columns)
            WgT_t = cpool.tile([128, 128], f32)
            WgA_t = cpool.tile([128, HC + 8], bf16)
            with tc.tile_pool(name="cpsum", bufs=1, space="PSUM") as cpsum:
                WgT_ps = cpsum.tile([128, 128], f32)
                nc.tensor.transpose(WgT_ps[:], Wg_t[:], eye_t[:])
                nc.vector.tensor_copy(WgT_t[:], WgT_ps[:])
                Wgatt_ps = cpsum.tile([128, 8], f32)
                nc.tensor.matmul(Wgatt_ps[:, 0:4], WgT_t[:], atts_t[:])
                nc.tensor.matmul(Wgatt_ps[:, 4:8], WgT_t[:], attd_t[:])
                nc.vector.tensor_copy(WgA_t[:, 0:HC], Wg_bf[:])
                nc.vector.tensor_copy(WgA_t[:, HC:HC + 8], Wgatt_ps[:])

            # ================= stage A =================
            with (
                tc.tile_pool(name="sa_sb", bufs=4) as sa,
                tc.tile_pool(name="sa_ps", bufs=3, space="PSUM") as saps,
                tc.tile_pool(name="sa_ps1", bufs=4, space="PSUM") as saps1,
            ):
                xT8 = None
                for g0 in range(0, TPC, 4):
                    gsz = min(4, TPC - g0)
                    gn = gsz * NT
                    if g0 % 8 == 0:
                        x8n = min(8, TPC - g0) * NT
                        xT8 = sa.tile([128, 5, 8 * NT], fp8, tag="xT", bufs=2)
                        nc.sync.dma_start(
                            xT8[:, :, 0:x8n],
                            xT_in.rearrange("(b p) n -> p b n", p=128)[
                                :, :, g0 * NT:g0 * NT + x8n])
                        xoff = 0
                    else:
                        xoff = 4 * NT
                    h1T_ps = saps.tile([128, 4 * NT], f32, tag="h1T")
                    for b in range(5):
                        nc.tensor.matmul(
                            h1T_ps[:, 0:gn],
                            fb_t[:, b, :],
                            xT8[:, b, xoff:xoff + gn],
                            start=(b == 0), stop=(b == 4))
                    h1T = sa.tile([128, 4 * NT], bf16, tag="h1Ts")
                    nc.vector.tensor_copy(h1T[:, 0:gn], h1T_ps[:, 0:gn])
                    hrow4 = sa.tile([128, 4, RW], fp8, tag="hrow4")
                    for u in range(gsz):
                        h_ps = saps1.tile([128, HC + 8], f32, tag="hps")
                        lhs = h1T[:, u * NT:(u + 1) * NT]
                        nc.tensor.matmul(h_ps[:, 0:HC + 8], lhs, WgA_t[:])
                        nc.vector.tensor_copy(hrow4[:, u, 0:HC],
                                              h_ps[:, 0:HC])
                        nc.scalar.activation(
                            hrow4[:, u, HC:HC + 16].bitcast(bf16),
                            h_ps[:, HC:HC + 8], AF.Copy)
                    nc.sync.dma_start(
                        Hext_loc[g0 * NT:g0 * NT + gn, 0:HC + 16].rearrange(
                            "(u p) c -> p u c", p=128),
                        hrow4[:, 0:gsz, 0:HC + 16])

                # AllGather with unmerged (opt=False) row-major APs.
                g = nc.gpsimd
                g.add_instruction(
                    mybir.InstCollectiveCompute(
                        name=f"I-{g.bass.next_id()}",
                        kind="AllGather",
                        op=mybir.AluOpType.bypass,
                        replica_groups=[core_ids],
                        ins=[g.lower_ap(Hext_loc[:], opt=False)],
                        outs=[g.lower_ap(Hfull[:], opt=False)],
                        unique_tensors="No",
                        cc_dim="Partition",
                    ))

            # ================= edge phase + MLP =================
            with (
                tc.tile_pool(name="eg_g", bufs=2) as egg,
                tc.tile_pool(name="eg_sb", bufs=3) as egs,
                tc.tile_pool(name="eg_ind", bufs=2) as egi,
                tc.tile_pool(name="eg_acc", bufs=3, space="PSUM") as egacc,
                tc.tile_pool(name="eg_tp", bufs=2, space="PSUM") as egtp,
                tc.tile_pool(name="mlp_sb", bufs=2) as msb,
                tc.tile_pool(name="mlp_ps", bufs=1, space="PSUM") as mps,
            ):
                actT4 = None
                mgsz = 4

                def issue_gathers(gidx):
                    a0, b0, e0 = (int(grp_a0[gidx]), int(grp_b0[gidx]),
                                  int(grp_end[gidx]))
                    nA, nB, nT_ = b0 - a0, e0 - b0, e0 - a0
                    isg = egg.tile([128, GT_MAX * 8], i16, tag="isg", bufs=3,
                                   name=f"isg_{gidx}")
                    nc.scalar.dma_start(isg[:, 0:nT_ * 8],
                                        idx_src[:, a0 * 8:e0 * 8])
                    adg = egg.tile([128, GT_MAX * 8], i16, tag="adg", bufs=3,
                                   name=f"adg_{gidx}")
                    nc.scalar.dma_start(adg[:, 0:nT_ * 8],
                                        idx_ad[:, a0 * 8:e0 * 8])
                    gAt = egg.tile([128, GA_MAX, HC + 8], fp8, tag="gA",
                                   bufs=3, name=f"gA_{gidx}")
                    small_gather(nc.gpsimd, gAt[:, 0:nA, :],
                                 Hfull[:, 0:HC + 8],
                                 isg[:, 0:nA * 8],
                                 num_idxs=nA * NT, elem_size=HC + 8,
                                 stride_b256=1)
                    adt = egg.tile([128, GT_MAX, 8], fp8, tag="adt", bufs=3,
                                   name=f"adt_{gidx}")
                    small_gather(nc.gpsimd, adt[:, 0:nT_, :],
                                 Hext_loc[:, HC + 8:HC + 16],
                                 adg[:, 0:nT_ * 8],
                                 num_idxs=nT_ * NT, elem_size=8, stride_b256=1)
                    gBt = egg.tile([128, GB_MAX, HC + 8], fp8, tag="gB",
                                   bufs=3, name=f"gB_{gidx}")
                    small_gather(nc.gpsimd, gBt[:, 0:nB, :],
                                 Hfull[BASE_B:NPAD, 0:HC + 8],
                                 isg[:, nA * 8:nT_ * 8],
                                 num_idxs=nB * NT, elem_size=HC + 8,
                                 stride_b256=1)
                    return gAt, gBt, adt

                pend = issue_gathers(0)
                for gidx in range(ngrp):
                    s0 = gidx * GRP
                    slots = list(range(s0, min(s0 + GRP, TPC)))
                    a0, b0, e0 = (int(grp_a0[gidx]), int(grp_b0[gidx]),
                                  int(grp_end[gidx]))
                    nA, nB, nT_ = b0 - a0, e0 - b0, e0 - a0
                    gAt, gBt, adt = pend
                    if gidx + 1 < ngrp:
                        pend = issue_gathers(gidx + 1)

                    # ---- per-region batched attention prep ----
                    gsA = egg.tile([128, GA_MAX, 136], bf16, tag="gsA",
                                   bufs=3, name=f"gsA_{gidx}")
                    gsB = egg.tile([128, GB_MAX, 136], bf16, tag="gsB",
                                   bufs=2, name=f"gsB_{gidx}")
                    regions = [(gAt, nA, 0, 0, gsA), (gBt, nB, b0 - a0, 1, gsB)]
                    inds = []
                    # phase 1: indicator builds (constants only) + a_s+a_d,
                    # prelu, exp for both regions
                    for (gt, nreg, adoff, r, gts) in regions:
                        ind = egi.tile(
                            [128, WSUM_A if r == 0 else WSUM_B], bf16,
                            tag="indA" if r == 0 else "indB",
                            name=f"ind_{gidx}_{r}")
                        inds.append(ind)
                        if nreg == 0:
                            continue
                        roff = a0 if r == 0 else b0
                        for (wl, r0, rl) in runs_grp[(gidx, r)]:
                            i0 = bass.AP(
                                iota_t.tensor, iota_t.offset,
                                [iota_t.ap[0], [0, rl], [2, wl // 2], [1, 2]])
                            i1 = bass.AP(
                                dcol_t.tensor,
                                dcol_t.offset + (roff + r0) * 2,
                                [dcol_t.ap[0], [2, rl], [0, wl // 2], [1, 2]])
                            o0 = bass.AP(
                                ind.tensor,
                                ind.offset + int(wcol[roff + r0]),
                                [ind.ap[0], [wl, rl], [2, wl // 2], [1, 2]])
                            nc.vector.tensor_tensor(o0, i0, i1, OP.is_equal)
                        # t = a_s + a_d (bf16 views of fp8 tiles)
                        ts = egs.tile([128, GA_MAX, 4], bf16, tag="ts",
                                      name=f"ts_{gidx}_{r}")
                        nc.vector.tensor_tensor(
                            ts[:, 0:nreg, :],
                            gt[:, 0:nreg, 128:136].bitcast(bf16),
                            adt[:, adoff:adoff + nreg, :].bitcast(bf16),
                            OP.add)
                        nc.scalar.activation(ts[:, 0:nreg, :], ts[:, 0:nreg, :],
                                             AF.Prelu, alpha=NEG_ATT)
                        # exp, duplicated pairs -> gts cols 128:136 (bf16)
                        ts_in = bass.AP(
                            ts.tensor, ts.offset,
                            [ts.ap[0], [4, nreg], [1, 4], [0, 2]])
                        ex_out = bass.AP(
                            gts.tensor, gts.offset + 128,
                            [gts.ap[0], [136, nreg], [1, 8]])
                        nc.scalar.activation(ex_out, ts_in, AF.Exp)
                    # phase 2: msg = h * ex (in0 fp8 h, in1 bf16 ex dup, out
                    # bf16); one head per region on gpsimd to balance DVE
                    for (gt, nreg, adoff, r, gts) in regions:
                        if nreg == 0:
                            continue
                        for h in range(4):
                            g4o = bass.AP(
                                gts.tensor, gts.offset + h * 32,
                                [gts.ap[0], [136, nreg], [2, 16], [1, 2]])
                            g4i = bass.AP(
                                gt.tensor, gt.offset + h * 32,
                                [gt.ap[0], [136, nreg], [2, 16], [1, 2]])
                            exb = bass.AP(
                                gts.tensor, gts.offset + 128 + h * 2,
                                [gts.ap[0], [136, nreg], [0, 16], [1, 2]])
                            eng = (nc.gpsimd if (h >= 3 or
                                   (h == 2 and r == 1)) else nc.vector)
                            eng.tensor_tensor(g4o, g4i, exb, OP.mult)

                    # ---- per-slot accumulate + finalize ----
                    for s in slots:
                        chunks = slot_chunks[s]
                        acc = egacc.tile([128, 136], f32, tag="acc")
                        first_full = chunks[0][4] == 128
                        if not first_full:
                            nc.tensor.matmul(acc[:], orow_t[:], zrow_t[:],
                                             start=True, stop=False,
                                             skip_group_check=True)
                        for i, (r, ci, wc, wo, wl) in enumerate(chunks):
                            gts = regions[r][4]
                            nc.tensor.matmul(
                                acc[wo:wo + wl, :],
                                inds[r][:, wc:wc + wl],
                                gts[:, ci, 0:136],
                                start=(i == 0 and first_full),
                                stop=(i == len(chunks) - 1),
                                skip_group_check=True)

                        # normalize + bias + ELU (node-major)
                        dinv = egs.tile([128, 4], f32, tag="dinv")
                        den = bass.AP(acc.tensor, acc.offset + 128,
                                      [acc.ap[0], [2, 4]])
                        nc.vector.reciprocal(dinv[:], den)
                        gat = egs.tile([128, 128], f32, tag="gat")
                        dinvb = bass.AP(dinv.tensor, dinv.offset,
                                        [dinv.ap[0], [1, 4], [0, 32]])
                        nc.vector.tensor_tensor(gat[:], acc[:, 0:128], dinvb,
                                                OP.mult)
                        # transpose; GAT bias becomes per-partition Act bias,
                        # ELU = relu(x) - relu(1 - exp(x)) folds it in.
                        sub = s % 4
                        if sub == 0:
                            mgsz = min(4, TPC - s)
                            actT4 = msb.tile([128, 4 * NT], bf16, tag="actT4")
                        tp = egtp.tile([128, 128], f32, tag="tp2", bufs=1)
                        nc.tensor.transpose(tp[:], gat[:], eye_t[:])
                        t1 = egs.tile([128, 128], f32, tag="t1")
                        nc.scalar.activation(t1[:], tp[:], AF.Exp,
                                             bias=bias_col[:, 0:1])
                        nc.scalar.activation(t1[:], t1[:], AF.Relu,
                                             scale=-1.0, bias=1.0)
                        rl_t = egs.tile([128, 128], f32, tag="rl")
                        nc.scalar.activation(rl_t[:], tp[:], AF.Relu,
                                             bias=bias_col[:, 0:1])
                        nc.vector.tensor_sub(actT4[:, sub * NT:(sub + 1) * NT],
                                             rl_t[:], t1[:])

                        if sub == mgsz - 1:
                            g0 = s - sub
                            gn = mgsz * NT
                            a1 = msb.tile([128, 2, 512], bf16, tag="a1")
                            for j in range(2):
                                o1 = mps.tile([128, 512], f32, tag="o1")
                                nc.tensor.matmul(
                                    o1[:, 0:gn],
                                    W1_t[:, j * 128:(j + 1) * 128],
                                    actT4[:, 0:gn])
                                nc.scalar.activation(
                                    a1[:, j, 0:gn], o1[:, 0:gn], AF.Prelu,
                                    alpha=NEG_MLP, bias=b1_t[:, j:j + 1])
                            o2 = mps.tile([128, 512], f32, tag="o2")
                            for j in range(2):
                                nc.tensor.matmul(
                                    o2[:, 0:gn], W2_t[:, j, :],
                                    a1[:, j, 0:gn],
                                    start=(j == 0), stop=(j == 1))
                            a2 = msb.tile([128, 512], bf16, tag="a2")
                            nc.scalar.activation(
                                a2[:, 0:gn], o2[:, 0:gn], AF.Prelu,
                                alpha=NEG_MLP, bias=b2_t[:])
                            o3 = mps.tile([16, 512], f32, tag="sm", name="o3_t")
                            nc.tensor.matmul(o3[0:10, 0:gn], W3_t[:],
                                             a2[:, 0:gn])
                            z = msb.tile([16, 512], bf16, tag="z")
                            nc.scalar.activation(
                                z[0:10, 0:gn], o3[0:10, 0:gn], AF.Prelu,
                                alpha=NEG_MLP, bias=b3_t[0:10, :])
                            nc.scalar.activation(z[0:10, 0:gn], z[0:10, 0:gn],
                                                 AF.Exp)
                            ssum = mps.tile([16, 512], f32, tag="sm",
                                            name="ssum_t")[0:1, :]
                            nc.tensor.matmul(
                                ssum[:, 0:gn], ones_t[0:10, 0:1],
                                z[0:10, 0:gn])
                            sinv = msb.tile([1, 512], bf16, tag="sinv")
                            with nc.allow_low_precision(
                                    reason="softmax denom bf16"):
                                nc.vector.reciprocal(sinv[:, 0:gn],
                                                     ssum[:, 0:gn])
                            sx = mps.tile([16, 512], f32, tag="sm", name="sx_t")
                            nc.tensor.matmul(
                                sx[0:10, 0:gn], ones_t[0:1, 0:10],
                                sinv[:, 0:gn])
                            res = msb.tile([16, 512], f32, tag="res")
                            nc.vector.tensor_mul(
                                res[0:10, 0:gn], z[0:10, 0:gn], sx[0:10, 0:gn])
                            nc.sync.dma_start(
                                outT[:, g0 * NT:g0 * NT + gn], res[0:10, 0:gn])

    nc.compile()
    return nc


def _inputs_per_core(inputs, src_w, ad_w, dcol2, node_order, meta):
    x = np.asarray(inputs["x"], dtype=np.float32)
    fb = np.asarray(inputs["fb"], dtype=np.float32)
    Wg = np.asarray(inputs["Wg"], dtype=np.float32)
    bias_g = np.asarray(inputs["bias_g"], dtype=np.float32)
    att_src = np.asarray(inputs["att_src"], dtype=np.float32)
    att_dst = np.asarray(inputs["att_dst"], dtype=np.float32)
    W1 = np.asarray(inputs["W1"], dtype=np.float32)
    b1 = np.asarray(inputs["b1"], dtype=np.float32)
    W2 = np.asarray(inputs["W2"], dtype=np.float32)
    b2 = np.asarray(inputs["b2"], dtype=np.float32)
    W3 = np.asarray(inputs["W3"], dtype=np.float32)
    b3 = np.asarray(inputs["b3"], dtype=np.float32)

    x_pad = np.zeros((NPAD, NFP), dtype=np.float32)
    x_pad[:N, :NF] = x
    fb_pad = np.zeros((NFP, NMEL), dtype=np.float32)
    fb_pad[:NF] = fb

    att_blk_s = np.zeros((HC, 4), dtype=np.float32)
    att_blk_d = np.zeros((HC, 4), dtype=np.float32)
    for h in range(H):
        att_blk_s[h * C:(h + 1) * C, h] = att_src[h]
        att_blk_d[h * C:(h + 1) * C, h] = att_dst[h]

    b1p = np.zeros((128, 2), dtype=np.float32)
    b1p[:, 0] = b1[:128]
    b1p[:, 1] = b1[128:]
    b2p = b2.reshape(128, 1).astype(np.float32)
    b3p = np.zeros((128, 1), dtype=np.float32)
    b3p[:10, 0] = b3

    iota_f32 = np.tile(np.arange(128, dtype=np.float32)[None, :], (128, 1))
    common = {
        "fb_p": _to_fp8(fb_pad), "Wg": Wg, "Wg_bf": _to_bf16(Wg),
        "attb_s": att_blk_s, "attb_d": att_blk_d,
        "bias_col": bias_g.reshape(128, 1).astype(np.float32),
        "W1": _to_bf16(W1), "b1": b1p, "W2": _to_bf16(W2), "b2": b2p,
        "W3": _to_bf16(W3), "b3": b3p,
        "eye": np.eye(128, dtype=np.float32),
        "iota": _to_bf16(iota_f32),
        "ones": _to_bf16(np.ones((128, 16), dtype=np.float32)),
    }

    maps = []
    for k in range(NCORES):
        m = dict(common)
        m["xT_in"] = _to_fp8(
            np.ascontiguousarray(x_pad[node_order[k]].T))
        m["idx_src"] = src_w[k]
        m["idx_ad"] = ad_w[k]
        m["dcol2"] = _to_bf16(dcol2[k].reshape(128, -1))
        maps.append(m)
    return maps


def kernel(**inputs):
    from concourse.bass_utils import run_bass_kernel_spmd

    src_w, ad_w, dcol2, node_order, meta = _prep(inputs["edge_index"])
    key = ("nc", meta["TOTC"], tuple(meta["cpt"].reshape(-1)),
           tuple(meta["woff"]))
    if key not in _CACHE:
        _CACHE.clear()
        _CACHE[key] = _build(meta)
    nc = _CACHE[key]
    maps = _inputs_per_core(inputs, src_w, ad_w, dcol2, node_order, meta)
    res = run_bass_kernel_spmd(nc, maps, core_ids=list(range(NCORES)))
    out = np.zeros((NPAD, 10), dtype=np.float32)
    for k in range(NCORES):
        out[node_order[k]] = res.results[k]["outT"].T
    return out[:N]


# revision 73
# speedup vs baseline: 1.0381x; 1.0090x over previous
"""GAT (gnn_message_passing) Trainium2 Bass kernel — 8-core SPMD.

Contract: kernel(**inputs) -> np.ndarray with FULL inputs / FULL output.
Self-contained: hardcodes shapes; only imports the container's concourse stack.

Design:
- Stage A: per-core h = (x @ fb) @ Wg (x/fb in fp8) plus attention dots ->
  local 256B node rows [h fp8(128) | a_s bf16(4) | a_d bf16(4) | pad].
- AllGather (unmerged row-structured APs) shares the node table.
- Edge phase, grouped by GRP destination slots per gather batch, with the
  next group's gathers issued one group ahead (software pipelining):
  * 136B/row gathers of source rows (h + a_s), in two int16 index halves,
  * an 8B/row gather of per-edge destination attention (a_d),
  * batched indicator builds and ex = exp(prelu(a_s+a_d)) with duplicated
    bf16 pairs so DVE ops hit the 2x perf mode,
  * msg = h * ex (fp8 x bf16 -> bf16; one head per region on gpsimd),
  * per-chunk indicator matmuls accumulate numerators+denominators in PSUM,
  * normalize (Act scale-copies), ELU via transpose, fused MLP + softmax.
- Destination tiles are assigned to (core, slot) by balanced ranking so the
  SPMD max-padding across cores stays small.
"""
import sys

for _p in ("/opt/trn_rl_repo", "/root/.axon_site/_ro/trn_rl_repo"):
    if _p not in sys.path:
        sys.path.append(_p)

import numpy as np

# ---------------- problem constants (hardcoded per contract) ----------------
N = 50000
NF = 513
NFP = 640            # padded feature dim (5 * 128)
NMEL = 128
H, C = 4, 32
HC = H * C           # 128
E = 800000
NEG_ATT = 0.2
NEG_MLP = 0.01

NCORES = 8
TPC = 49             # tiles (slots) per core
NT = 128             # nodes per tile
NPC = TPC * NT       # 6272 nodes per core
NPAD = NCORES * NPC  # 50176
NTILES_G = NPAD // NT  # 392
RW = 256             # Hfull row elems (bf16 -> 512 B)
ADR = 128            # adrep row elems (bf16 -> 256 B row stride)
SPLIT = 32768        # max int16 gather index + 1
BASE_B = NPAD - SPLIT  # 17408; group-B gathers read Hfull[BASE_B:]
GRP = 6              # slots per gather group

_CACHE = {}


def _to_fp8(a):
    import ml_dtypes
    return np.asarray(a, dtype=np.float32).astype(ml_dtypes.float8_e4m3fn)


def _to_bf16(a):
    """f32 -> bf16 (round-to-nearest-even)."""
    try:
        import ml_dtypes
        return np.asarray(a, dtype=np.float32).astype(ml_dtypes.bfloat16)
    except ImportError:
        x = np.ascontiguousarray(a, dtype=np.float32).view(np.uint32)
        rounded = (((x >> 16) + ((x >> 15) & 1)) & 0xFFFF).astype(np.uint16)
        return rounded


def _prep(edge_index):
    """Host-side edge preprocessing. Returns per-core index/metadata arrays."""
    src = np.asarray(edge_index[0], dtype=np.int64)
    dst = np.asarray(edge_index[1], dtype=np.int64)
    loop = np.arange(N, dtype=np.int64)
    src = np.concatenate([src, loop])
    dst = np.concatenate([dst, loop])

    # ---- balanced tile -> (core, slot) assignment ----
    tile_of = dst // NT
    tcnt = np.bincount(tile_of, minlength=NTILES_G)
    order = np.argsort(-tcnt, kind="stable")      # rank r -> tile
    # slot r//8 gets ranks r with r%8 as core
    tile_core = np.empty(NTILES_G, dtype=np.int64)
    tile_slot = np.empty(NTILES_G, dtype=np.int64)
    for r, t in enumerate(order):
        tile_slot[t] = r // NCORES
        tile_core[t] = r % NCORES

    # node -> permuted global row
    nodes = np.arange(NPAD, dtype=np.int64)
    n_tile = nodes // NT
    perm_row = tile_core[n_tile] * NPC + tile_slot[n_tile] * NT + nodes % NT
    # node_order[k, local] = node id
    node_order = np.empty((NCORES, NPC), dtype=np.int64)
    node_order[perm_row // NPC, perm_row % NPC] = nodes

    # ---- per-edge core/slot/half ----
    ecore = tile_core[tile_of]
    eslot = tile_slot[tile_of]
    prow = perm_row[src]
    half = (prow >= SPLIT).astype(np.int64)
    srow = np.where(half == 1, prow - BASE_B, prow)
    dloc = dst % NT

    # sort per (core, slot, half, dst)
    so = np.lexsort((src, dloc, half, eslot, ecore))
    ecore, eslot, half, srow, dloc = (a[so] for a in (ecore, eslot, half, srow, dloc))

    # counts per (core, slot, half)
    cnt = np.zeros((NCORES, TPC, 2), dtype=np.int64)
    np.add.at(cnt, (ecore, eslot, half), 1)
    starts = np.zeros((NCORES, TPC, 2), dtype=np.int64)
    starts.reshape(-1)[1:] = np.cumsum(cnt.reshape(-1))[:-1]

    # chunks per (slot, half): max over cores
    cpt = np.maximum(1, -(-cnt.max(axis=0) // NT))   # [TPC, 2]

    ngrp = -(-TPC // GRP)
    # chunk order: per group g: A-chunks of slots g*4.., then B-chunks
    slot_chunk_off = np.zeros((TPC, 2), dtype=np.int64)   # abs chunk idx of each (s,half) run
    grp_a0 = np.zeros(ngrp, dtype=np.int64)
    grp_b0 = np.zeros(ngrp, dtype=np.int64)
    grp_end = np.zeros(ngrp, dtype=np.int64)
    co = 0
    for g in range(ngrp):
        slots = range(g * GRP, min((g + 1) * GRP, TPC))
        grp_a0[g] = co
        for s in slots:
            slot_chunk_off[s, 0] = co
            co += cpt[s, 0]
        grp_b0[g] = co
        for s in slots:
            slot_chunk_off[s, 1] = co
            co += cpt[s, 1]
        grp_end[g] = co
    TOTC = co
    TOTIDX = TOTC * NT

    # fill per-chunk per-core tables (chunk-major layout)
    src_rel = np.zeros((NCORES, TOTC, NT), dtype=np.int64)
    ad_idx = np.zeros((NCORES, TOTC, NT), dtype=np.int64)
    dloc_all = np.zeros((NCORES, TOTC, NT), dtype=np.int64)
    valid = np.zeros((NCORES, TOTC, NT), dtype=bool)
    for k in range(NCORES):
        for s in range(TPC):
            for hf in range(2):
                c0 = slot_chunk_off[s, hf]
                nch = int(cpt[s, hf])
                st, cn = starts[k, s, hf], int(cnt[k, s, hf])
                src_rel[k, c0:c0 + nch].reshape(-1)[:cn] = srow[st:st + cn]
                ad_idx[k, c0:c0 + nch].reshape(-1)[:cn] = (
                    eslot[st:st + cn] * NT + dloc[st:st + cn])
                dloc_all[k, c0:c0 + nch].reshape(-1)[:cn] = dloc[st:st + cn]
                valid[k, c0:c0 + nch].reshape(-1)[:cn] = True

    assert src_rel.min() >= 0 and src_rel.max() < SPLIT

    # windows per chunk (union over cores): narrowest of 32 at 32-aligned
    # offset <= 64, else 64 at {0, 64}, else full 128. PSUM matmul outputs
    # may start at partitions {0, 32, 64}.
    woff = np.zeros(TOTC, dtype=np.int64)
    wlen = np.full(TOTC, 128, dtype=np.int64)
    for c in range(TOTC):
        v = valid[:, c, :]
        if v.any():
            dl = dloc_all[:, c, :][v]
            lo, hi = int(dl.min()), int(dl.max())
            wo32 = min((lo // 32) * 32, 64)
            if hi < wo32 + 32:
                woff[c] = wo32
                wlen[c] = 32
            else:
                wo = 0 if lo < 64 else 64
                if hi < wo + 64:
                    woff[c] = wo
                    wlen[c] = 64

    # chunk -> owning slot (before region reorder)
    slot_of_chunk = np.zeros(TOTC, dtype=np.int64)
    for s in range(TPC):
        for hf in range(2):
            c0, nch = int(slot_chunk_off[s, hf]), int(cpt[s, hf])
            slot_of_chunk[c0:c0 + nch] = s

    # reorder chunks within each group REGION by wlen desc (stable) so one
    # batched indicator build covers each wl class; track per-slot lists.
    runs_grp = {}
    for g in range(ngrp):
        for r, (r0, r1) in enumerate(((int(grp_a0[g]), int(grp_b0[g])),
                                      (int(grp_b0[g]), int(grp_end[g])))):
            sl = slice(r0, r1)
            ro = np.argsort(-wlen[sl], kind="stable")
            for arr in (woff, wlen, slot_of_chunk):
                arr[sl] = arr[sl][ro]
            for arr in (src_rel, ad_idx, dloc_all, valid):
                arr[:, sl] = arr[:, sl][:, ro]
            rl = []
            i = r0
            while i < r1:
                w = int(wlen[i])
                j = i
                while j < r1 and int(wlen[j]) == w:
                    j += 1
                rl.append((w, i - r0, j - i))
                i = j
            runs_grp[(g, r)] = rl

    # per-slot chunk lists: (region, rel_idx, ind_col, woff, wlen)
    slot_chunks = {s: [] for s in range(TPC)}
    wcol = np.zeros(TOTC, dtype=np.int64)   # ind-arena col offset per chunk
    WSUM_A = WSUM_B = 0
    for g in range(ngrp):
        for r, (r0, r1) in enumerate(((int(grp_a0[g]), int(grp_b0[g])),
                                      (int(grp_b0[g]), int(grp_end[g])))):
            off = 0
            for c in range(r0, r1):
                wcol[c] = off
                off += int(wlen[c])
            if r == 0:
                WSUM_A = max(WSUM_A, off)
            else:
                WSUM_B = max(WSUM_B, off)
            for c in range(r0, r1):
                s = int(slot_of_chunk[c])
                slot_chunks[s].append((r, c - r0, int(wcol[c]), int(woff[c]),
                                       int(wlen[c])))
    for s in range(TPC):
        slot_chunks[s].sort(key=lambda t: -t[4])

    dst_rel = np.full((NCORES, TOTC, NT), 999.0, dtype=np.float32)
    for k in range(NCORES):
        dr = dloc_all[k] - woff[None, :].T
        dst_rel[k][valid[k]] = dr[valid[k]].astype(np.float32)

    # wrapped int16 index layout: [128, TOTIDX//16]
    def wrap(a):
        fl = a.reshape(NCORES, TOTIDX)
        w = fl.reshape(NCORES, TOTIDX // 16, 16).transpose(0, 2, 1)
        return np.tile(w, (1, 8, 1)).astype(np.int16)

    src_w = wrap(src_rel)
    ad_w = wrap(ad_idx)
    # dcol duplicated pairs, bf16: [NCORES, 128, TOTC, 2]
    dcol2 = np.repeat(dst_rel.transpose(0, 2, 1)[:, :, :, None], 2, axis=3)

    meta = {"cpt": cpt, "woff": woff, "wlen": wlen, "TOTC": TOTC,
            "TOTIDX": TOTIDX, "runs_grp": runs_grp, "wcol": wcol,
            "slot_chunks": slot_chunks, "slot_off": slot_chunk_off,
            "grp_a0": grp_a0, "grp_b0": grp_b0, "grp_end": grp_end,
            "ngrp": ngrp, "WSUM_A": WSUM_A, "WSUM_B": WSUM_B}
    return src_w, ad_w, dcol2, node_order, meta


def _build(meta):
    import concourse.bass as bass
    import concourse.bacc as bacc
    import concourse.mybir as mybir
    import concourse.tile as tile

    f32 = mybir.dt.float32
    bf16 = mybir.dt.bfloat16
    fp8 = mybir.dt.float8e4
    i16 = mybir.dt.int16
    AF = mybir.ActivationFunctionType
    OP = mybir.AluOpType

    cpt, woff, wlen = meta["cpt"], meta["woff"], meta["wlen"]
    TOTC, TOTIDX = meta["TOTC"], meta["TOTIDX"]
    runs_grp, slot_off = meta["runs_grp"], meta["slot_off"]
    slot_chunks, wcol = meta["slot_chunks"], meta["wcol"]
    grp_a0, grp_b0, grp_end = meta["grp_a0"], meta["grp_b0"], meta["grp_end"]
    ngrp = meta["ngrp"]

    GA_MAX = int(max(int(grp_b0[g] - grp_a0[g]) for g in range(ngrp)))
    WSUM_A, WSUM_B = meta["WSUM_A"], meta["WSUM_B"]
    GB_MAX = int(max(int(grp_end[g] - grp_b0[g]) for g in range(ngrp)))
    GT_MAX = int(max(int(grp_end[g] - grp_a0[g]) for g in range(ngrp)))
    TOT_MAX = int((cpt[:, 0] + cpt[:, 1]).max())

    nc = bacc.Bacc("TRN2", target_bir_lowering=False, debug=False)

    # ---- I/O ----
    xT_in = nc.dram_tensor("xT_in", [NFP, NPC], fp8, kind="ExternalInput")
    idx_src = nc.dram_tensor("idx_src", [128, TOTIDX // 16], i16, kind="ExternalInput")
    idx_ad = nc.dram_tensor("idx_ad", [128, TOTIDX // 16], i16, kind="ExternalInput")
    dcol_d = nc.dram_tensor("dcol2", [128, TOTC * 2], bf16, kind="ExternalInput")
    fb_p = nc.dram_tensor("fb_p", [NFP, NMEL], fp8, kind="ExternalInput")
    Wg_d = nc.dram_tensor("Wg", [NMEL, HC], f32, kind="ExternalInput")
    Wg_bf_d = nc.dram_tensor("Wg_bf", [NMEL, HC], bf16, kind="ExternalInput")
    attb_s = nc.dram_tensor("attb_s", [HC, 4], f32, kind="ExternalInput")
    attb_d = nc.dram_tensor("attb_d", [HC, 4], f32, kind="ExternalInput")
    bias_col_d = nc.dram_tensor("bias_col", [128, 1], f32, kind="ExternalInput")
    W1_d = nc.dram_tensor("W1", [HC, 256], bf16, kind="ExternalInput")
    b1_d = nc.dram_tensor("b1", [128, 2], f32, kind="ExternalInput")
    W2_d = nc.dram_tensor("W2", [256, HC], bf16, kind="ExternalInput")
    b2_d = nc.dram_tensor("b2", [128, 1], f32, kind="ExternalInput")
    W3_d = nc.dram_tensor("W3", [HC, 10], bf16, kind="ExternalInput")
    b3_d = nc.dram_tensor("b3", [128, 1], f32, kind="ExternalInput")
    eye_d = nc.dram_tensor("eye", [128, 128], f32, kind="ExternalInput")
    iota_d = nc.dram_tensor("iota", [128, 128], bf16, kind="ExternalInput")
    ones_d = nc.dram_tensor("ones", [128, 16], bf16, kind="ExternalInput")
    outT = nc.dram_tensor("outT", [10, NPC], f32, kind="ExternalOutput")

    core_ids = list(range(NCORES))

    def small_gather(g, out_ap, in_ap, idxs_ap, num_idxs, elem_size,
                     stride_b256):
        """Hand-rolled InstDMAGatherAnt allowing elem < 256B (stride stays a
        256B multiple, which is the actual ISA constraint)."""
        _in_ap = g.lower_ap_dma(in_ap, for_custom_bir_dma=True)
        return g.add_instruction(
            mybir.InstDMAGatherAnt(
                name=g.bass.get_next_instruction_name(),
                ins=[*_in_ap, g.lower_ap(idxs_ap),
                     g.lower_val_access(g.to_reg(num_idxs))],
                outs=[g.lower_ap(out_ap)],
                transpose=False,
                num_idxs=num_idxs,
                elem_size=elem_size,
                stride_bytes_256=stride_b256,
                gen_mode=0,
                single_packet=False,
                queue_num=0,
                sbuf_tokens_per_rank=0,
                sbuf_free_dim_per_rank=0,
                sbuf_free_dim_pad_per_rank=0,
                sbuf_byte_offset=0,
            ))

    with tile.TileContext(nc) as tc:
        with (
            tc.tile_pool(name="dram", bufs=1, space="DRAM") as dpool,
            tc.tile_pool(name="const", bufs=1) as cpool,
        ):
            Hext_loc = dpool.tile([NPC, RW], fp8)
            Hfull = dpool.tile([NPAD, RW], fp8, addr_space="Shared")

            # ---- constants to SBUF ----
            fb_t = cpool.tile([128, 5, NMEL], fp8)
            nc.sync.dma_start(fb_t[:], fb_p.rearrange("(b p) m -> p b m", p=128))
            Wg_t = cpool.tile([128, HC], f32)
            nc.sync.dma_start(Wg_t[:], Wg_d[:])
            Wg_bf = cpool.tile([128, HC], bf16)
            nc.sync.dma_start(Wg_bf[:], Wg_bf_d[:])
            atts_t = cpool.tile([128, 4], f32)
            nc.sync.dma_start(atts_t[:], attb_s[:])
            attd_t = cpool.tile([128, 4], f32)
            nc.sync.dma_start(attd_t[:], attb_d[:])
            bias_col = cpool.tile([128, 1], f32)
            nc.sync.dma_start(bias_col[:], bias_col_d[:])
            W1_t = cpool.tile([128, 256], bf16)
            nc.sync.dma_start(W1_t[:], W1_d[:])
            b1_t = cpool.tile([128, 2], f32)
            nc.sync.dma_start(b1_t[:], b1_d[:])
            W2_t = cpool.tile([128, 2, HC], bf16)
            nc.sync.dma_start(W2_t[:], W2_d.rearrange("(b p) m -> p b m", p=128))
            b2_t = cpool.tile([128, 1], f32)
            nc.sync.dma_start(b2_t[:], b2_d[:])
            W3_t = cpool.tile([128, 10], bf16)
            nc.sync.dma_start(W3_t[:], W3_d[:])
            b3_t = cpool.tile([128, 1], f32)
            nc.sync.dma_start(b3_t[:], b3_d[:])
            eye_t = cpool.tile([128, 128], f32)
            nc.sync.dma_start(eye_t[:], eye_d[:])
            zrow_t = cpool.tile([1, 136], bf16)
            nc.vector.memset(zrow_t[:], 0.0)
            orow_t = cpool.tile([1, 128], bf16)
            nc.vector.memset(orow_t[:], 1.0)
            iota_t = cpool.tile([128, 128], bf16)
            nc.sync.dma_start(iota_t[:], iota_d[:])
            ones_t = cpool.tile([128, 16], bf16)
            nc.sync.dma_start(ones_t[:], ones_d[:])
            dcol_t = cpool.tile([128, TOTC * 2], bf16)
            nc.scalar.dma_start(dcol_t[:], dcol_d[:])

            # WgT, Wgatt (h -> [h | a_s | a_d] projection columns)
            WgT_t = cpool.tile([128, 128], f32)
            WgA_t = cpool.tile([128, HC + 8], bf16)
            with tc.tile_pool(name="cpsum", bufs=1, space="PSUM") as cpsum:
                WgT_ps = cpsum.tile([128, 128], f32)
                nc.tensor.transpose(WgT_ps[:], Wg_t[:], eye_t[:])
                nc.vector.tensor_copy(WgT_t[:], WgT_ps[:])
                Wgatt_ps = cpsum.tile([128, 8], f32)
                nc.tensor.matmul(Wgatt_ps[:, 0:4], WgT_t[:], atts_t[:])
                nc.tensor.matmul(Wgatt_ps[:, 4:8], WgT_t[:], attd_t[:])
                nc.vector.tensor_copy(WgA_t[:, 0:HC], Wg_bf[:])
                nc.vector.tensor_copy(WgA_t[:, HC:HC + 8], Wgatt_ps[:])

            # ================= stage A =================
            with (
                tc.tile_pool(name="sa_sb", bufs=4) as sa,
                tc.tile_pool(name="sa_ps", bufs=3, space="PSUM") as saps,
                tc.tile_pool(name="sa_ps1", bufs=4, space="PSUM") as saps1,
            ):
                xT8 = None
                for g0 in range(0, TPC, 4):
                    gsz = min(4, TPC - g0)
                    gn = gsz * NT
                    if g0 % 8 == 0:
                        x8n = min(8, TPC - g0) * NT
                        xT8 = sa.tile([128, 5, 8 * NT], fp8, tag="xT", bufs=2)
                        nc.sync.dma_start(
                            xT8[:, :, 0:x8n],
                            xT_in.rearrange("(b p) n -> p b n", p=128)[
                                :, :, g0 * NT:g0 * NT + x8n])
                        xoff = 0
                    else:
                        xoff = 4 * NT
                    h1T_ps = saps.tile([128, 4 * NT], f32, tag="h1T")
                    for b in range(5):
                        nc.tensor.matmul(
                            h1T_ps[:, 0:gn],
                            fb_t[:, b, :],
                            xT8[:, b, xoff:xoff + gn],
                            start=(b == 0), stop=(b == 4))
                    h1T = sa.tile([128, 4 * NT], bf16, tag="h1Ts")
                    nc.scalar.activation(h1T[:, 0:gn], h1T_ps[:, 0:gn],
                                         AF.Copy)
                    hrow4 = sa.tile([128, 4, RW], fp8, tag="hrow4")
                    for u in range(gsz):
                        h_ps = saps1.tile([128, HC + 8], f32, tag="hps")
                        lhs = h1T[:, u * NT:(u + 1) * NT]
                        nc.tensor.matmul(h_ps[:, 0:HC + 8], lhs, WgA_t[:])
                        nc.vector.tensor_copy(hrow4[:, u, 0:HC],
                                              h_ps[:, 0:HC])
                        nc.scalar.activation(
                            hrow4[:, u, HC:HC + 16].bitcast(bf16),
                            h_ps[:, HC:HC + 8], AF.Copy)
                    nc.sync.dma_start(
                        Hext_loc[g0 * NT:g0 * NT + gn, 0:HC + 16].rearrange(
                            "(u p) c -> p u c", p=128),
                        hrow4[:, 0:gsz, 0:HC + 16])

                # AllGather with unmerged (opt=False) row-major APs.
                g = nc.gpsimd
                g.add_instruction(
                    mybir.InstCollectiveCompute(
                        name=f"I-{g.bass.next_id()}",
                        kind="AllGather",
                        op=mybir.AluOpType.bypass,
                        replica_groups=[core_ids],
                        ins=[g.lower_ap(Hext_loc[:], opt=False)],
                        outs=[g.lower_ap(Hfull[:], opt=False)],
                        unique_tensors="No",
                        cc_dim="Partition",
                    ))

            # ================= edge phase + MLP =================
            with (
                tc.tile_pool(name="eg_g", bufs=2) as egg,
                tc.tile_pool(name="eg_sb", bufs=3) as egs,
                tc.tile_pool(name="eg_ind", bufs=2) as egi,
                tc.tile_pool(name="eg_acc", bufs=3, space="PSUM") as egacc,
                tc.tile_pool(name="eg_tp", bufs=2, space="PSUM") as egtp,
                tc.tile_pool(name="mlp_sb", bufs=2) as msb,
                tc.tile_pool(name="mlp_ps", bufs=1, space="PSUM") as mps,
            ):
                actT4 = None
                mgsz = 4

                def issue_gathers(gidx):
                    a0, b0, e0 = (int(grp_a0[gidx]), int(grp_b0[gidx]),
                                  int(grp_end[gidx]))
                    nA, nB, nT_ = b0 - a0, e0 - b0, e0 - a0
                    isg = egg.tile([128, GT_MAX * 8], i16, tag="isg", bufs=3,
                                   name=f"isg_{gidx}")
                    nc.scalar.dma_start(isg[:, 0:nT_ * 8],
                                        idx_src[:, a0 * 8:e0 * 8])
                    adg = egg.tile([128, GT_MAX * 8], i16, tag="adg", bufs=3,
                                   name=f"adg_{gidx}")
                    nc.scalar.dma_start(adg[:, 0:nT_ * 8],
                                        idx_ad[:, a0 * 8:e0 * 8])
                    gAt = egg.tile([128, GA_MAX, HC + 8], fp8, tag="gA",
                                   bufs=3, name=f"gA_{gidx}")
                    small_gather(nc.gpsimd, gAt[:, 0:nA, :],
                                 Hfull[:, 0:HC + 8],
                                 isg[:, 0:nA * 8],
                                 num_idxs=nA * NT, elem_size=HC + 8,
                                 stride_b256=1)
                    adt = egg.tile([128, GT_MAX, 8], fp8, tag="adt", bufs=3,
                                   name=f"adt_{gidx}")
                    small_gather(nc.gpsimd, adt[:, 0:nT_, :],
                                 Hext_loc[:, HC + 8:HC + 16],
                                 adg[:, 0:nT_ * 8],
                                 num_idxs=nT_ * NT, elem_size=8, stride_b256=1)
                    gBt = egg.tile([128, GB_MAX, HC + 8], fp8, tag="gB",
                                   bufs=3, name=f"gB_{gidx}")
                    small_gather(nc.gpsimd, gBt[:, 0:nB, :],
                                 Hfull[BASE_B:NPAD, 0:HC + 8],
                                 isg[:, nA * 8:nT_ * 8],
                                 num_idxs=nB * NT, elem_size=HC + 8,
                                 stride_b256=1)
                    return gAt, gBt, adt

                pend = issue_gathers(0)
                for gidx in range(ngrp):
                    s0 = gidx * GRP
                    slots = list(range(s0, min(s0 + GRP, TPC)))
                    a0, b0, e0 = (int(grp_a0[gidx]), int(grp_b0[gidx]),
                                  int(grp_end[gidx]))
                    nA, nB, nT_ = b0 - a0, e0 - b0, e0 - a0
                    gAt, gBt, adt = pend
                    if gidx + 1 < ngrp:
                        pend = issue_gathers(gidx + 1)

                    # ---- per-region batched attention prep ----
                    gsA = egg.tile([128, GA_MAX, 136], bf16, tag="gsA",
                                   bufs=3, name=f"gsA_{gidx}")
                    gsB = egg.tile([128, GB_MAX, 136], bf16, tag="gsB",
                                   bufs=2, name=f"gsB_{gidx}")
                    regions = [(gAt, nA, 0, 0, gsA), (gBt, nB, b0 - a0, 1, gsB)]
                    inds = []
                    # phase 1: indicator builds (constants only) + a_s+a_d,
                    # prelu, exp for both regions
                    for (gt, nreg, adoff, r, gts) in regions:
                        ind = egi.tile(
                            [128, WSUM_A if r == 0 else WSUM_B], bf16,
                            tag="indA" if r == 0 else "indB",
                            name=f"ind_{gidx}_{r}")
                        inds.append(ind)
                        if nreg == 0:
                            continue
                        roff = a0 if r == 0 else b0
                        for (wl, r0, rl) in runs_grp[(gidx, r)]:
                            i0 = bass.AP(
                                iota_t.tensor, iota_t.offset,
                                [iota_t.ap[0], [0, rl], [2, wl // 2], [1, 2]])
                            i1 = bass.AP(
                                dcol_t.tensor,
                                dcol_t.offset + (roff + r0) * 2,
                                [dcol_t.ap[0], [2, rl], [0, wl // 2], [1, 2]])
                            o0 = bass.AP(
                                ind.tensor,
                                ind.offset + int(wcol[roff + r0]),
                                [ind.ap[0], [wl, rl], [2, wl // 2], [1, 2]])
                            nc.vector.tensor_tensor(o0, i0, i1, OP.is_equal)
                        # t = a_s + a_d (bf16 views of fp8 tiles)
                        ts = egs.tile([128, GA_MAX, 4], bf16, tag="ts",
                                      name=f"ts_{gidx}_{r}")
                        nc.vector.tensor_tensor(
                            ts[:, 0:nreg, :],
                            gt[:, 0:nreg, 128:136].bitcast(bf16),
                            adt[:, adoff:adoff + nreg, :].bitcast(bf16),
                            OP.add)
                        nc.scalar.activation(ts[:, 0:nreg, :], ts[:, 0:nreg, :],
                                             AF.Prelu, alpha=NEG_ATT)
                        # exp, duplicated pairs -> gts cols 128:136 (bf16)
                        ts_in = bass.AP(
                            ts.tensor, ts.offset,
                            [ts.ap[0], [4, nreg], [1, 4], [0, 2]])
                        ex_out = bass.AP(
                            gts.tensor, gts.offset + 128,
                            [gts.ap[0], [136, nreg], [1, 8]])
                        nc.scalar.activation(ex_out, ts_in, AF.Exp)
                    # phase 2: msg = h * ex (in0 fp8 h, in1 bf16 ex dup, out
                    # bf16); one head per region on gpsimd to balance DVE
                    for (gt, nreg, adoff, r, gts) in regions:
                        if nreg == 0:
                            continue
                        for h in range(4):
                            g4o = bass.AP(
                                gts.tensor, gts.offset + h * 32,
                                [gts.ap[0], [136, nreg], [2, 16], [1, 2]])
                            g4i = bass.AP(
                                gt.tensor, gt.offset + h * 32,
                                [gt.ap[0], [136, nreg], [2, 16], [1, 2]])
                            exb = bass.AP(
                                gts.tensor, gts.offset + 128 + h * 2,
                                [gts.ap[0], [136, nreg], [0, 16], [1, 2]])
                            eng = (nc.gpsimd if (h >= 3 or
                                   (h == 2 and r == 1)) else nc.vector)
                            eng.tensor_tensor(g4o, g4i, exb, OP.mult)

                    # ---- per-slot accumulate + finalize ----
                    for s in slots:
                        chunks = slot_chunks[s]
                        acc = egacc.tile([128, 136], f32, tag="acc")
                        first_full = chunks[0][4] == 128
                        if not first_full:
                            nc.tensor.matmul(acc[:], orow_t[:], zrow_t[:],
                                             start=True, stop=False,
                                             skip_group_check=True)
                        for i, (r, ci, wc, wo, wl) in enumerate(chunks):
                            gts = regions[r][4]
                            nc.tensor.matmul(
                                acc[wo:wo + wl, :],
                                inds[r][:, wc:wc + wl],
                                gts[:, ci, 0:136],
                                start=(i == 0 and first_full),
                                stop=(i == len(chunks) - 1),
                                skip_group_check=True)

                        # normalize + bias + ELU (node-major)
                        dinv = egs.tile([128, 4], f32, tag="dinv")
                        den = bass.AP(acc.tensor, acc.offset + 128,
                                      [acc.ap[0], [2, 4]])
                        nc.vector.reciprocal(dinv[:], den)
                        gat = egs.tile([128, 128], f32, tag="gat")
                        dinvb = bass.AP(dinv.tensor, dinv.offset,
                                        [dinv.ap[0], [1, 4], [0, 32]])
                        nc.vector.tensor_tensor(gat[:], acc[:, 0:128], dinvb,
                                                OP.mult)
                        # transpose; GAT bias becomes per-partition Act bias,
                        # ELU = relu(x) - relu(1 - exp(x)) folds it in.
                        sub = s % 4
                        if sub == 0:
                            mgsz = min(4, TPC - s)
                            actT4 = msb.tile([128, 4 * NT], bf16, tag="actT4")
                        tp = egtp.tile([128, 128], f32, tag="tp2", bufs=1)
                        nc.tensor.transpose(tp[:], gat[:], eye_t[:])
                        t1 = egs.tile([128, 128], f32, tag="t1")
                        nc.scalar.activation(t1[:], tp[:], AF.Exp,
                                             bias=bias_col[:, 0:1])
                        nc.scalar.activation(t1[:], t1[:], AF.Relu,
                                             scale=-1.0, bias=1.0)
                        rl_t = egs.tile([128, 128], f32, tag="rl")
                        nc.scalar.activation(rl_t[:], tp[:], AF.Relu,
                                             bias=bias_col[:, 0:1])
                        nc.vector.tensor_sub(actT4[:, sub * NT:(sub + 1) * NT],
                                             rl_t[:], t1[:])

                        if sub == mgsz - 1:
                            g0 = s - sub
                            gn = mgsz * NT
                            a1 = msb.tile([128, 2, 512], bf16, tag="a1")
                            for j in range(2):
                                o1 = mps.tile([128, 512], f32, tag="o1")
                                nc.tensor.matmul(
                                    o1[:, 0:gn],
                                    W1_t[:, j * 128:(j + 1) * 128],
                                    actT4[:, 0:gn])
                                nc.scalar.activation(
                                    a1[:, j, 0:gn], o1[:, 0:gn], AF.Prelu,
                                    alpha=NEG_MLP, bias=b1_t[:, j:j + 1])
                            o2 = mps.tile([128, 512], f32, tag="o2")
                            for j in range(2):
                                nc.tensor.matmul(
                                    o2[:, 0:gn], W2_t[:, j, :],
                                    a1[:, j, 0:gn],
                                    start=(j == 0), stop=(j == 1))
                            a2 = msb.tile([128, 512], bf16, tag="a2")
                            nc.scalar.activation(
                                a2[:, 0:gn], o2[:, 0:gn], AF.Prelu,
                                alpha=NEG_MLP, bias=b2_t[:])
                            o3 = mps.tile([16, 512], f32, tag="sm", name="o3_t")
                            nc.tensor.matmul(o3[0:10, 0:gn], W3_t[:],
                                             a2[:, 0:gn])
                            z = msb.tile([16, 512], bf16, tag="z")
                            nc.scalar.activation(
                                z[0:10, 0:gn], o3[0:10, 0:gn], AF.Prelu,
                                alpha=NEG_MLP, bias=b3_t[0:10, :])
                            nc.scalar.activation(z[0:10, 0:gn], z[0:10, 0:gn],
                                                 AF.Exp)
                            ssum = mps.tile([16, 512], f32, tag="sm",
                                            name="ssum_t")[0:1, :]
                            nc.tensor.matmul(
                                ssum[:, 0:gn], ones_t[0:10, 0:1],
                                z[0:10, 0:gn])
                            sinv = msb.tile([1, 512], bf16, tag="sinv")
                            with nc.allow_low_precision(
                                    reason="softmax denom bf16"):
                                nc.vector.reciprocal(sinv[:, 0:gn],
                                                     ssum[:, 0:gn])
                            sx = mps.tile([16, 512], f32, tag="sm", name="sx_t")
                            nc.tensor.matmul(
                                sx[0:10, 0:gn], ones_t[0:1, 0:10],
                                sinv[:, 0:gn])
                            res = msb.tile([16, 512], f32, tag="res")
                            nc.vector.tensor_mul(
                                res[0:10, 0:gn], z[0:10, 0:gn], sx[0:10, 0:gn])
                            nc.sync.dma_start(
                                outT[:, g0 * NT:g0 * NT + gn], res[0:10, 0:gn])

    nc.compile()
    return nc


def _inputs_per_core(inputs, src_w, ad_w, dcol2, node_order, meta):
    x = np.asarray(inputs["x"], dtype=np.float32)
    fb = np.asarray(inputs["fb"], dtype=np.float32)
    Wg = np.asarray(inputs["Wg"], dtype=np.float32)
    bias_g = np.asarray(inputs["bias_g"], dtype=np.float32)
    att_src = np.asarray(inputs["att_src"], dtype=np.float32)
    att_dst = np.asarray(inputs["att_dst"], dtype=np.float32)
    W1 = np.asarray(inputs["W1"], dtype=np.float32)
    b1 = np.asarray(inputs["b1"], dtype=np.float32)
    W2 = np.asarray(inputs["W2"], dtype=np.float32)
    b2 = np.asarray(inputs["b2"], dtype=np.float32)
    W3 = np.asarray(inputs["W3"], dtype=np.float32)
    b3 = np.asarray(inputs["b3"], dtype=np.float32)

    x_pad = np.zeros((NPAD, NFP), dtype=np.float32)
    x_pad[:N, :NF] = x
    fb_pad = np.zeros((NFP, NMEL), dtype=np.float32)
    fb_pad[:NF] = fb

    att_blk_s = np.zeros((HC, 4), dtype=np.float32)
    att_blk_d = np.zeros((HC, 4), dtype=np.float32)
    for h in range(H):
        att_blk_s[h * C:(h + 1) * C, h] = att_src[h]
        att_blk_d[h * C:(h + 1) * C, h] = att_dst[h]

    b1p = np.zeros((128, 2), dtype=np.float32)
    b1p[:, 0] = b1[:128]
    b1p[:, 1] = b1[128:]
    b2p = b2.reshape(128, 1).astype(np.float32)
    b3p = np.zeros((128, 1), dtype=np.float32)
    b3p[:10, 0] = b3

    iota_f32 = np.tile(np.arange(128, dtype=np.float32)[None, :], (128, 1))
    common = {
        "fb_p": _to_fp8(fb_pad), "Wg": Wg, "Wg_bf": _to_bf16(Wg),
        "attb_s": att_blk_s, "attb_d": att_blk_d,
        "bias_col": bias_g.reshape(128, 1).astype(np.float32),
        "W1": _to_bf16(W1), "b1": b1p, "W2": _to_bf16(W2), "b2": b2p,
        "W3": _to_bf16(W3), "b3": b3p,
        "eye": np.eye(128, dtype=np.float32),
        "iota": _to_bf16(iota_f32),
        "ones": _to_bf16(np.ones((128, 16), dtype=np.float32)),
    }

    maps = []
    for k in range(NCORES):
        m = dict(common)
        m["xT_in"] = _to_fp8(
            np.ascontiguousarray(x_pad[node_order[k]].T))
        m["idx_src"] = src_w[k]
        m["idx_ad"] = ad_w[k]
        m["dcol2"] = _to_bf16(dcol2[k].reshape(128, -1))
        maps.append(m)
    return maps


def kernel(**inputs):
    from concourse.bass_utils import run_bass_kernel_spmd

    src_w, ad_w, dcol2, node_order, meta = _prep(inputs["edge_index"])
    key = ("nc", meta["TOTC"], tuple(meta["cpt"].reshape(-1)),
           tuple(meta["woff"]))
    if key not in _CACHE:
        _CACHE.clear()
        _CACHE[key] = _build(meta)
    nc = _CACHE[key]
    maps = _inputs_per_core(inputs, src_w, ad_w, dcol2, node_order, meta)
    res = run_bass_kernel_spmd(nc, maps, core_ids=list(range(NCORES)))
    out = np.zeros((NPAD, 10), dtype=np.float32)
    for k in range(NCORES):
        out[node_order[k]] = res.results[k]["outT"].T
    return out[:N]
